# revision 6
# baseline (speedup 1.0000x reference)
"""Trainium2 Bass kernel for MoE routing (2-layer expert MLP + softmax).

Strategy: expert-parallel across the 8 NeuronCores. The reference computes
all 8 experts for every sample and then gathers the one selected by
`domain`; mathematically only the selected expert's MLP matters per sample.
The host groups samples by expert, core e receives only the ~B/8 samples
routed to expert e (padded to a uniform per-core capacity so all cores run
the same SPMD program) plus expert e's weights. Each core runs a dense
2-layer MLP + softmax in a transposed layout:

    hT[f2, n]  = relu(W1[:, f2].T @ xT[:, n] + b1[f2])   (PE + ACT)
    lT[c, n]   = W2[:, c].T @ hT[:, n]                   (PE)
    expT       = exp(lT + b2)                            (ACT)
    sT[c, n]   = ones[C,C].T @ expT                      (PE partition sum)
    out[c, n]  = expT * (1 / sT)                         (DVE)

All matmul operands are bfloat16 (PSUM accumulation stays fp32).

v2 timeline notes (from the v1 NTFF trace, core with max count 2104):
- The framework preamble ends ~6.3us; nothing the kernel does can start
  earlier. The framework epilogue costs ~3.5us after the last store lands.
- The MM stream already ran at the measured roofline (N/2.4GHz + 2.5ns
  per matmul, LDWEIGHTS fully hidden at N>=256), so v2 attacks the edges:
  * v1's first real matmul waited until 13.3us for w1's first m-block,
    which sat on the scalar ring BEHIND the b1/b2 bias transfers - tiny
    but descriptor-dense (128 x 16B / 64 x 4B descriptors ~ 2.6us of DMA
    engine grind). v2 bakes both biases INTO the weight tensors as two
    trailing bf16 columns per block (fp32 bit pattern split across two
    bf16 slots, read back via AP.bitcast(f32) - bit-exact) so the weight
    stream starts immediately and there are no fine-grained transfers.
  * v1 chunked the batch [512 x 4, 56]; the 56-col tail chunk paid 37
    LDWEIGHTS-exposed matmuls (~100ns each vs the 25ns floor, ~2.8us
    wasted). v2 chunks [256, ~462 x 4]: every matmul's moving dim is
    >=256 so LDWEIGHTS stays hidden, and the small FIRST chunk needs only
    512KB of x before the real stream can start.
  * Ring ownership instead of halving every transfer: scalar (HWDGE,
    fastest starter) carries w1 m0 then x0's k5:8 tail then w1 m1..m3/w2
    interleaved between chunk-0 waves; sync (HWDGE) carries x0 k0:5 and
    the k5:8 share of every later chunk (+ output stores); gpsimd (SWDGE,
    ~2.2us descriptor-gen lag but ~285GB/s once streaming) starts on
    chunk 1's k0:5 share immediately and owns that share for all later
    chunks. Consumers only wait on DMAs emitted earlier on the same ring,
    so each chunk's triggers are emitted right before/after the waves
    that need them, exactly as in v1.
- Warmup: the HAM clock gate cannot reach K=8/8 before first-busy+3.4us,
  and cold matmuls still retire work at half rate, so the warmup burst is
  sized only to bridge engine-boot (~6.85us) to x0/w1m0-ready (~9.5us);
  real waves start cold and warm up in place.
- exp tiles are two persistent buffers zeroed once during the fill (rows
  C..127 must be zero for the ones-matmul partition sum); v1 re-memset
  them every chunk.
"""

import math
from collections import deque

import ml_dtypes
import numpy as np

import concourse.bacc as bacc
import concourse.bass as bass
import concourse.mybir as mybir
import concourse.tile as tile
from concourse.bass import ds
from concourse.bass_utils import run_bass_kernel_spmd

N_CORES = 8
BF16 = ml_dtypes.bfloat16

K_SPLIT = 5  # x k-blocks 0:5 on sync/gpsimd, 5:8 on scalar/sync
WU_BIG = 5  # 512-col warmup matmuls (cold, ~427ns each)
WU_SMALL = 4  # 128-col warmup matmuls (fine-grained handoff)

_program_cache: dict[tuple, object] = {}


def _chunk_sizes(cap: int) -> list[int]:
    """[256, then balanced <=512 chunks]."""
    if cap <= 512:
        return [cap]
    c0 = 256
    rest = cap - c0
    n = math.ceil(rest / 512)
    base, r = divmod(rest, n)
    return [c0] + [base + (1 if i < r else 0) for i in range(n)]


def _build_program(cap: int, F1: int, F2: int, C: int):
    key = (cap, F1, F2, C)
    if key in _program_cache:
        return _program_cache[key]

    assert F1 % 128 == 0 and F2 % 128 == 0
    K1 = F1 // 128
    M1 = F2 // 128
    K2 = F2 // 128
    assert C <= 128

    f32 = mybir.dt.float32
    bf16 = mybir.dt.bfloat16
    nc = bacc.Bacc(None, target_bir_lowering=False, debug=False)

    chunks = _chunk_sizes(cap)
    offs = [0]
    for cn in chunks:
        offs.append(offs[-1] + cn)
    n_chunks = len(chunks)

    x_d = [
        nc.dram_tensor(f"xt{ci}", [128, K1, cn], bf16, kind="ExternalInput")
        for ci, cn in enumerate(chunks)
    ]
    # w1: per m-block, K1 x 128 weight columns + 2 bf16 columns holding the
    # fp32 bias bit pattern (read via bitcast - exact).
    w1_d = nc.dram_tensor("w1", [128, M1, K1 * 128 + 2], bf16, kind="ExternalInput")
    # w2: K2 x 128 weight columns (output padded to 128: 64-wide matmuls
    # run ~1.5x slower) + 2 bias columns.
    w2_d = nc.dram_tensor("w2", [128, K2 * 128 + 2], bf16, kind="ExternalInput")
    out_d = nc.dram_tensor("outT", [C, cap], bf16, kind="ExternalOutput")

    with tile.TileContext(nc) as tc:
        with (
            tc.tile_pool(name="const", bufs=1) as const_pool,
            tc.tile_pool(name="expp", bufs=2) as e_pool,
            tc.tile_pool(name="xin", bufs=2) as x_pool,
            tc.tile_pool(name="h", bufs=2 * M1 + 2) as h_pool,
            tc.tile_pool(name="out", bufs=2) as o_pool,
            tc.tile_pool(name="rec", bufs=2) as r_pool,
            tc.tile_pool(name="ph", bufs=5, space="PSUM") as ph_pool,
            tc.tile_pool(name="pl", bufs=2, space="PSUM") as pl_pool,
            tc.tile_pool(name="pb", bufs=1, space="PSUM") as pb_pool,
        ):
            # Scalar ring (HWDGE): w1 m0 first - nothing descriptor-dense
            # ahead of it - then the x0 k5:8 tail. w1 m1..m3 and w2 are
            # emitted between chunk-0 waves below.
            w1_sb = const_pool.tile([128, M1, K1 * 128 + 2], bf16)
            nc.scalar.dma_start(w1_sb[:, 0, :], w1_d[:, 0, :])
            w2_sb = const_pool.tile([128, K2 * 128 + 2], bf16)

            # Sync ring (HWDGE): x0 k0:5.
            xt = []
            t = x_pool.tile([128, K1, chunks[0]], bf16, tag="xt", name="xt0")
            nc.sync.dma_start(t[:, :K_SPLIT, :], x_d[0][:, :K_SPLIT, :])
            nc.scalar.dma_start(t[:, K_SPLIT:, :], x_d[0][:, K_SPLIT:, :])
            xt.append(t)

            # GpSimd ring (SWDGE): warmup operand memset first (the first
            # warmup matmul waits on it), then chunk 1's k0:5 share so its
            # ~2.2us descriptor-gen lag overlaps chunk 0's fill, then the
            # constants that aren't needed until chunk 0's softmax stages.
            wu_x = const_pool.tile([128, 512], bf16)
            nc.gpsimd.memset(wu_x[:], 0.0)
            if n_chunks > 1:
                t = x_pool.tile([128, K1, chunks[1]], bf16, tag="xt", name="xt1")
                nc.gpsimd.dma_start(t[:, :K_SPLIT, :], x_d[1][:, :K_SPLIT, :])
                xt.append(t)
            # ones[128, 128] all-ones stationary for the partition sum
            # (contraction and output dims padded to 128; exp rows C..127
            # are zero so the padding adds nothing).
            ones_cc = const_pool.tile([128, 128], bf16)
            nc.gpsimd.memset(ones_cc[:], 1.0)
            # Two persistent exp tiles, fully zeroed ONCE here; the EXP
            # activation only ever writes rows 0:C so rows C..127 stay 0.
            max_cn = max(chunks)
            exp_tiles = []
            for i in range(2):
                e = e_pool.tile([128, max_cn], bf16, tag="expt", name=f"exp{i}")
                nc.gpsimd.memset(e[:], 0.0)
                exp_tiles.append(e)

            # Warmup: bridge engine-boot (~6.85us) to first-data (~9.5us).
            # Cold matmuls retire real work at half rate, so undershoot
            # beats overshoot; HAM goes warm at first-busy+3.4us no matter
            # what we do here.
            for i in range(WU_BIG):
                wu_ps = ph_pool.tile([128, 512], f32, tag="ph", name=f"wu{i}")
                nc.tensor.matmul(
                    wu_ps[:], wu_x[:, :128], wu_x[:], start=True, stop=True
                )
            for i in range(WU_SMALL):
                wu_ps = ph_pool.tile([128, 128], f32, tag="ph", name=f"wv{i}")
                nc.tensor.matmul(
                    wu_ps[:], wu_x[:, :128], wu_x[:, :128], start=True, stop=True
                )

            b1_ap = [
                w1_sb[:, m, K1 * 128 : K1 * 128 + 2].bitcast(f32)
                for m in range(M1)
            ]
            b2_ap = w2_sb[0:C, K2 * 128 : K2 * 128 + 2].bitcast(f32)

            stages: deque = deque()

            def stage_l2(ci: int, cn: int, ht: list):
                pl = pl_pool.tile([128, cn], f32, tag="pl")
                for k in range(K2):
                    nc.tensor.matmul(
                        pl[:],
                        w2_sb[:, k * 128 : (k + 1) * 128],
                        ht[k][:],
                        start=(k == 0),
                        stop=(k == K2 - 1),
                    )
                expt = exp_tiles[ci % 2]
                nc.scalar.activation(
                    expt[0:C, :cn],
                    pl[0:C, :],
                    mybir.ActivationFunctionType.Exp,
                    bias=b2_ap,
                )
                stages.append(lambda: stage_norm(ci, cn, expt))

            def stage_norm(ci: int, cn: int, expt):
                pb = pb_pool.tile([128, cn], f32, tag="pb")
                nc.tensor.matmul(
                    pb[:], ones_cc[:], expt[:, :cn], start=True, stop=True
                )
                rec = r_pool.tile([C, cn], f32, tag="rec")
                nc.vector.reciprocal_approx_fast(rec[:], pb[0:C, :])
                ot = o_pool.tile([C, cn], bf16, tag="ot")
                nc.vector.tensor_mul(ot[:], expt[0:C, :cn], rec[:])
                # Sync HWDGE: SWDGE descriptor generation costs ~1us and
                # the final store sits on the critical tail.
                nc.sync.dma_start(out_d[:, ds(offs[ci], cn)], ot[:])

            for ci, cn in enumerate(chunks):
                ht = []
                for m in range(M1):
                    ph = ph_pool.tile([128, cn], f32, tag="ph")
                    for k in range(K1):
                        nc.tensor.matmul(
                            ph[:],
                            w1_sb[:, m, k * 128 : (k + 1) * 128],
                            xt[ci][:, k, :],
                            start=(k == 0),
                            stop=(k == K1 - 1),
                        )
                    hm = h_pool.tile([128, cn], bf16, tag="ht")
                    nc.scalar.activation(
                        hm[:],
                        ph[:],
                        mybir.ActivationFunctionType.Relu,
                        bias=b1_ap[m],
                    )
                    ht.append(hm)
                    if ci == 0 and m + 1 < M1:
                        # Emitted AFTER wave m so wave m+1 (not wave m)
                        # carries the wait for this transfer.
                        nc.scalar.dma_start(
                            w1_sb[:, m + 1, :], w1_d[:, m + 1, :]
                        )
                    if ci == 0 and m == 2:
                        nc.scalar.dma_start(w2_sb[:], w2_d[:])
                    if stages:
                        stages.popleft()()
                # Next chunk's x, emitted AFTER this chunk's waves so this
                # chunk's matmuls don't wait on it (a consumer waits on
                # every DMA emitted earlier on the same ring). Chunk ci+1's
                # gpsimd k0:5 share (chunk 1's was emitted at the head);
                # the k5:8 share rides the sync ring behind the stores.
                nxt = ci + 1
                if nxt < n_chunks:
                    cnn = chunks[nxt]
                    if nxt == 1:
                        t = xt[1]
                    else:
                        t = x_pool.tile(
                            [128, K1, cnn], bf16, tag="xt", name=f"xt{nxt}"
                        )
                        nc.gpsimd.dma_start(
                            t[:, :K_SPLIT, :], x_d[nxt][:, :K_SPLIT, :]
                        )
                        xt.append(t)
                    nc.sync.dma_start(
                        t[:, K_SPLIT:, :], x_d[nxt][:, K_SPLIT:, :]
                    )
                stages.append(lambda ci=ci, cn=cn, ht=ht: stage_l2(ci, cn, ht))
            while stages:
                stages.popleft()()

    nc.compile()
    _program_cache[key] = nc
    return nc


def _pack_bias_cols(b: np.ndarray) -> np.ndarray:
    """fp32 [..., n] -> bf16 [..., n, 2] bit-pattern split."""
    a = np.ascontiguousarray(b, dtype="<f4")
    return a.view(np.uint16).view(BF16).reshape(*a.shape, 2)


def kernel(domain, x, W1, b1, W2, b2):
    domain = np.asarray(domain)
    x = np.ascontiguousarray(np.asarray(x, dtype=np.float32))
    W1 = np.asarray(W1, dtype=np.float32)
    b1 = np.asarray(b1, dtype=np.float32)
    W2 = np.asarray(W2, dtype=np.float32)
    b2 = np.asarray(b2, dtype=np.float32)

    B, F1 = x.shape
    E, _, F2 = W1.shape
    C = W2.shape[2]
    K1 = F1 // 128
    K2 = F2 // 128
    M1 = F2 // 128
    assert E == N_CORES

    xb = x.astype(BF16)
    W1b = W1.astype(BF16)
    W2b = W2.astype(BF16)

    idx = [np.nonzero(domain == e)[0] for e in range(E)]
    counts = [len(i) for i in idx]
    cap = max(512, max(counts))
    chunks = _chunk_sizes(cap)

    nc = _build_program(cap, F1, F2, C)

    in_maps = []
    for e in range(E):
        xT = np.zeros((F1, cap), BF16)
        xT[:, : counts[e]] = xb[idx[e]].T
        # [F1, cap] -> [128, K1, cap] SBUF tile layout.
        xT4 = xT.reshape(K1, 128, cap).transpose(1, 0, 2)

        w1p = np.zeros((128, M1, K1 * 128 + 2), BF16)
        w1p[:, :, : K1 * 128] = (
            W1b[e].reshape(K1, 128, M1, 128).transpose(1, 2, 0, 3)
            .reshape(128, M1, K1 * 128)
        )
        # b1 [F2] -> [128 partitions (f2-within), M1] fp32 bits.
        w1p[:, :, K1 * 128 :] = _pack_bias_cols(b1[e].reshape(M1, 128).T)

        w2p = np.zeros((128, K2 * 128 + 2), BF16)
        w2pad = np.zeros((128, K2, 128), BF16)
        w2pad[:, :, :C] = W2b[e].reshape(K2, 128, C).transpose(1, 0, 2)
        w2p[:, : K2 * 128] = w2pad.reshape(128, K2 * 128)
        w2p[:C, K2 * 128 :] = _pack_bias_cols(b2[e])

        m = {
            "w1": np.ascontiguousarray(w1p),
            "w2": np.ascontiguousarray(w2p),
        }
        n0 = 0
        for ci, cn in enumerate(chunks):
            m[f"xt{ci}"] = np.ascontiguousarray(xT4[:, :, n0 : n0 + cn])
            n0 += cn
        in_maps.append(m)

    res = run_bass_kernel_spmd(nc, in_maps, core_ids=list(range(N_CORES)))

    out = np.empty((B, C), np.float32)
    for e in range(E):
        out[idx[e]] = res.results[e]["outT"][:, : counts[e]].T.astype(np.float32)
    return out


# revision 10
# speedup vs baseline: 1.0245x; 1.0245x over previous
"""Trainium2 Bass kernel for MoE routing (2-layer expert MLP + softmax).

Strategy: expert-parallel across the 8 NeuronCores. The reference computes
all 8 experts for every sample and then gathers the one selected by
`domain`; mathematically only the selected expert's MLP matters per sample.
The host groups samples by expert, core e receives only the ~B/8 samples
routed to expert e (padded to a uniform per-core capacity so all cores run
the same SPMD program) plus expert e's weights. Each core runs a dense
2-layer MLP + softmax in a transposed layout:

    hT[f2, n]  = relu(W1[:, f2].T @ xT[:, n] + b1[f2])   (PE + ACT)
    lT[c, n]   = W2[:, c].T @ hT[:, n]                   (PE)
    expT       = exp(lT + b2)                            (ACT)
    sT[c, n]   = ones[C,C].T @ expT                      (PE partition sum)
    out[c, n]  = expT * (1 / sT)                         (DVE)

All matmul operands are bfloat16 (PSUM accumulation stays fp32).

v2 timeline notes (from the v1 NTFF trace, core with max count 2104):
- The framework preamble ends ~6.3us; nothing the kernel does can start
  earlier. The framework epilogue costs ~3.5us after the last store lands.
- The MM stream already ran at the measured roofline (N/2.4GHz + 2.5ns
  per matmul, LDWEIGHTS fully hidden at N>=256), so v2 attacks the edges:
  * v1's first real matmul waited until 13.3us for w1's first m-block,
    which sat on the scalar ring BEHIND the b1/b2 bias transfers - tiny
    but descriptor-dense (128 x 16B / 64 x 4B descriptors ~ 2.6us of DMA
    engine grind). v2 bakes both biases INTO the weight tensors as two
    trailing bf16 columns per block (fp32 bit pattern split across two
    bf16 slots, read back via AP.bitcast(f32) - bit-exact) so the weight
    stream starts immediately and there are no fine-grained transfers.
  * v1 chunked the batch [512 x 4, 56]; the 56-col tail chunk paid 37
    LDWEIGHTS-exposed matmuls (~100ns each vs the 25ns floor, ~2.8us
    wasted). v2 chunks [256, ~462 x 4]: every matmul's moving dim is
    >=256 so LDWEIGHTS stays hidden, and the small FIRST chunk needs only
    512KB of x before the real stream can start.
  * Ring ownership instead of halving every transfer: scalar (HWDGE,
    fastest starter) carries w1 m0 then x0's k5:8 tail then w1 m1..m3/w2
    interleaved between chunk-0 waves; sync (HWDGE) carries x0 k0:5 and
    the k5:8 share of every later chunk (+ output stores); gpsimd (SWDGE,
    ~2.2us descriptor-gen lag but ~285GB/s once streaming) starts on
    chunk 1's k0:5 share immediately and owns that share for all later
    chunks. Consumers only wait on DMAs emitted earlier on the same ring,
    so each chunk's triggers are emitted right before/after the waves
    that need them, exactly as in v1.
- Warmup: the HAM clock gate cannot reach K=8/8 before first-busy+3.4us,
  and cold matmuls still retire work at half rate, so the warmup burst is
  sized only to bridge engine-boot (~6.85us) to x0/w1m0-ready (~9.5us);
  real waves start cold and warm up in place.
- exp tiles are two persistent buffers zeroed once during the fill (rows
  C..127 must be zero for the ones-matmul partition sum); v1 re-memset
  them every chunk.
"""

import math
from collections import deque

import ml_dtypes
import numpy as np

import concourse.bacc as bacc
import concourse.bass as bass
import concourse.mybir as mybir
import concourse.tile as tile
from concourse.bass import ds
from concourse.bass_utils import run_bass_kernel_spmd

N_CORES = 8
BF16 = ml_dtypes.bfloat16

K_SPLIT = 5  # x k-blocks 0:5 on sync (~190GB/s), 5:8 on scalar (~330GB/s)
WU_BIG = 8  # 512-col warmup matmuls (cold, ~427ns each)
WU_SMALL = 4  # 128-col warmup matmuls (fine-grained handoff)

_program_cache: dict[tuple, object] = {}


def _chunk_sizes(cap: int) -> list[int]:
    """[balanced <=512 chunks..., 256]: big chunks first (per-core DMA is
    only ~250-430GB/s aggregate, so the front must not need w1+x0+x1 all
    at once - a big chunk 0 buys the conveyor time), small chunk LAST so
    the post-stream tail (exp/norm/store) is short."""
    if cap <= 512:
        return [cap]
    cl = 256
    rest = cap - cl
    n = math.ceil(rest / 512)
    base, r = divmod(rest, n)
    return [base + (1 if i < r else 0) for i in range(n)] + [cl]


def _build_program(cap: int, F1: int, F2: int, C: int):
    key = (cap, F1, F2, C)
    if key in _program_cache:
        return _program_cache[key]

    assert F1 % 128 == 0 and F2 % 128 == 0
    K1 = F1 // 128
    M1 = F2 // 128
    K2 = F2 // 128
    assert C <= 128

    f32 = mybir.dt.float32
    bf16 = mybir.dt.bfloat16
    nc = bacc.Bacc(None, target_bir_lowering=False, debug=False)

    chunks = _chunk_sizes(cap)
    offs = [0]
    for cn in chunks:
        offs.append(offs[-1] + cn)
    n_chunks = len(chunks)

    x_d = [
        nc.dram_tensor(f"xt{ci}", [128, K1, cn], bf16, kind="ExternalInput")
        for ci, cn in enumerate(chunks)
    ]
    # w1: per m-block, K1 x 128 weight columns + 2 bf16 columns holding the
    # fp32 bias bit pattern (read via bitcast - exact).
    w1_d = nc.dram_tensor("w1", [128, M1, K1 * 128 + 2], bf16, kind="ExternalInput")
    # w2: K2 x 128 weight columns (output padded to 128: 64-wide matmuls
    # run ~1.5x slower) + 2 bias columns.
    w2_d = nc.dram_tensor("w2", [128, K2 * 128 + 2], bf16, kind="ExternalInput")
    out_d = nc.dram_tensor("outT", [C, cap], bf16, kind="ExternalOutput")

    with tile.TileContext(nc) as tc:
        with (
            tc.tile_pool(name="const", bufs=1) as const_pool,
            tc.tile_pool(name="expp", bufs=2) as e_pool,
            tc.tile_pool(name="xin", bufs=2) as x_pool,
            tc.tile_pool(name="h", bufs=2 * M1 + 2) as h_pool,
            tc.tile_pool(name="out", bufs=2) as o_pool,
            tc.tile_pool(name="rec", bufs=2) as r_pool,
            tc.tile_pool(name="ph", bufs=5, space="PSUM") as ph_pool,
            tc.tile_pool(name="pl", bufs=2, space="PSUM") as pl_pool,
            tc.tile_pool(name="pb", bufs=1, space="PSUM") as pb_pool,
        ):
            # Scalar ring (HWDGE): w1 m0 first - nothing descriptor-dense
            # ahead of it - then the x0 k5:8 tail. w1 m1..m3 and w2 are
            # emitted between chunk-0 waves below.
            w1_sb = const_pool.tile([128, M1, K1 * 128 + 2], bf16)
            nc.scalar.dma_start(w1_sb[:, 0, :], w1_d[:, 0, :])
            w2_sb = const_pool.tile([128, K2 * 128 + 2], bf16)

            # Sync ring (HWDGE, ~190GB/s but first data ~1.5us after
            # trigger): x0 k0:5. Scalar follows w1m0 with x0's k5:8 tail;
            # same-ring jobs serialize in order, so the two rings form two
            # parallel need-ordered conveyors with no cross-traffic.
            xt = []
            t = x_pool.tile([128, K1, chunks[0]], bf16, tag="xt", name="xt0")
            nc.sync.dma_start(t[:, :K_SPLIT, :], x_d[0][:, :K_SPLIT, :])
            nc.scalar.dma_start(t[:, K_SPLIT:, :], x_d[0][:, K_SPLIT:, :])
            xt.append(t)

            # GpSimd (SWDGE) does NO early DMA: its descriptor generation
            # interleaving with the HWDGE streams collapsed aggregate DMA
            # bandwidth to ~190GB/s in v2. It only carries x3/x4 k0:5
            # shares late (emitted after chunk 1/2's waves, gated by the
            # x-pool WAR) when the PE is busy and the fill is over.
            wu_x = const_pool.tile([128, 512], bf16)
            nc.gpsimd.memset(wu_x[:], 0.0)
            # ones[128, 128] all-ones stationary for the partition sum
            # (contraction and output dims padded to 128; exp rows C..127
            # are zero so the padding adds nothing).
            ones_cc = const_pool.tile([128, 128], bf16)
            nc.gpsimd.memset(ones_cc[:], 1.0)
            # Two persistent exp tiles, fully zeroed ONCE here; the EXP
            # activation only ever writes rows 0:C so rows C..127 stay 0.
            max_cn = max(chunks)
            exp_tiles = []
            for i in range(2):
                e = e_pool.tile([128, max_cn], bf16, tag="expt", name=f"exp{i}")
                nc.gpsimd.memset(e[:], 0.0)
                exp_tiles.append(e)

            # Warmup: bridge engine-boot (~6.85us) to first-data (~9.5us).
            # Cold matmuls retire real work at half rate, so undershoot
            # beats overshoot; HAM goes warm at first-busy+3.4us no matter
            # what we do here.
            for i in range(WU_BIG):
                wu_ps = ph_pool.tile([128, 512], f32, tag="ph", name=f"wu{i}")
                nc.tensor.matmul(
                    wu_ps[:], wu_x[:, :128], wu_x[:], start=True, stop=True
                )
            for i in range(WU_SMALL):
                wu_ps = ph_pool.tile([128, 128], f32, tag="ph", name=f"wv{i}")
                nc.tensor.matmul(
                    wu_ps[:], wu_x[:, :128], wu_x[:, :128], start=True, stop=True
                )

            b1_ap = [
                w1_sb[:, m, K1 * 128 : K1 * 128 + 2].bitcast(f32)
                for m in range(M1)
            ]
            b2_ap = w2_sb[0:C, K2 * 128 : K2 * 128 + 2].bitcast(f32)

            stages: deque = deque()

            def stage_l2(ci: int, cn: int, ht: list):
                pl = pl_pool.tile([128, cn], f32, tag="pl")
                for k in range(K2):
                    nc.tensor.matmul(
                        pl[:],
                        w2_sb[:, k * 128 : (k + 1) * 128],
                        ht[k][:],
                        start=(k == 0),
                        stop=(k == K2 - 1),
                    )
                expt = exp_tiles[ci % 2]
                nc.scalar.activation(
                    expt[0:C, :cn],
                    pl[0:C, :],
                    mybir.ActivationFunctionType.Exp,
                    bias=b2_ap,
                )
                stages.append(lambda: stage_norm(ci, cn, expt))

            def stage_norm(ci: int, cn: int, expt):
                pb = pb_pool.tile([128, cn], f32, tag="pb")
                nc.tensor.matmul(
                    pb[:], ones_cc[:], expt[:, :cn], start=True, stop=True
                )
                rec = r_pool.tile([C, cn], f32, tag="rec")
                nc.vector.reciprocal_approx_fast(rec[:], pb[0:C, :])
                ot = o_pool.tile([C, cn], bf16, tag="ot")
                nc.vector.tensor_mul(ot[:], expt[0:C, :cn], rec[:])
                # Sync HWDGE: SWDGE descriptor generation costs ~1us and
                # the final store sits on the critical tail.
                nc.sync.dma_start(out_d[:, ds(offs[ci], cn)], ot[:])

            for ci, cn in enumerate(chunks):
                ht = []
                for m in range(M1):
                    ph = ph_pool.tile([128, cn], f32, tag="ph")
                    for k in range(K1):
                        nc.tensor.matmul(
                            ph[:],
                            w1_sb[:, m, k * 128 : (k + 1) * 128],
                            xt[ci][:, k, :],
                            start=(k == 0),
                            stop=(k == K1 - 1),
                        )
                    hm = h_pool.tile([128, cn], bf16, tag="ht")
                    nc.scalar.activation(
                        hm[:],
                        ph[:],
                        mybir.ActivationFunctionType.Relu,
                        bias=b1_ap[m],
                    )
                    ht.append(hm)
                    if ci == 0 and m + 1 < M1:
                        # Emitted AFTER wave m so wave m+1 (not wave m)
                        # carries the wait for this transfer.
                        nc.scalar.dma_start(
                            w1_sb[:, m + 1, :], w1_d[:, m + 1, :]
                        )
                    if ci == 0 and m == 2:
                        nc.scalar.dma_start(w2_sb[:], w2_d[:])
                    if stages:
                        stages.popleft()()
                # Next chunk's x, emitted AFTER this chunk's waves so this
                # chunk's matmuls don't wait on it (a consumer waits on
                # every DMA emitted earlier on the same ring). x1/x2 ride
                # the two HWDGE conveyors right behind the fill; x3/x4
                # move their k0:5 bulk to gpsimd (by then its SWDGE lag is
                # harmless and the HWDGE rings carry stores + k5:8).
                nxt = ci + 1
                if nxt < n_chunks:
                    cnn = chunks[nxt]
                    t = x_pool.tile(
                        [128, K1, cnn], bf16, tag="xt", name=f"xt{nxt}"
                    )
                    bulk = nc.sync if nxt <= 2 else nc.gpsimd
                    bulk.dma_start(t[:, :K_SPLIT, :], x_d[nxt][:, :K_SPLIT, :])
                    tail_eng = nc.scalar if nxt <= 2 else nc.sync
                    tail_eng.dma_start(
                        t[:, K_SPLIT:, :], x_d[nxt][:, K_SPLIT:, :]
                    )
                    xt.append(t)
                stages.append(lambda ci=ci, cn=cn, ht=ht: stage_l2(ci, cn, ht))
            while stages:
                stages.popleft()()

    nc.compile()
    _program_cache[key] = nc
    return nc


def _pack_bias_cols(b: np.ndarray) -> np.ndarray:
    """fp32 [..., n] -> bf16 [..., n, 2] bit-pattern split."""
    a = np.ascontiguousarray(b, dtype="<f4")
    return a.view(np.uint16).view(BF16).reshape(*a.shape, 2)


def kernel(domain, x, W1, b1, W2, b2):
    domain = np.asarray(domain)
    x = np.ascontiguousarray(np.asarray(x, dtype=np.float32))
    W1 = np.asarray(W1, dtype=np.float32)
    b1 = np.asarray(b1, dtype=np.float32)
    W2 = np.asarray(W2, dtype=np.float32)
    b2 = np.asarray(b2, dtype=np.float32)

    B, F1 = x.shape
    E, _, F2 = W1.shape
    C = W2.shape[2]
    K1 = F1 // 128
    K2 = F2 // 128
    M1 = F2 // 128
    assert E == N_CORES

    xb = x.astype(BF16)
    W1b = W1.astype(BF16)
    W2b = W2.astype(BF16)

    idx = [np.nonzero(domain == e)[0] for e in range(E)]
    counts = [len(i) for i in idx]
    cap = max(512, max(counts))
    chunks = _chunk_sizes(cap)

    nc = _build_program(cap, F1, F2, C)

    in_maps = []
    for e in range(E):
        xT = np.zeros((F1, cap), BF16)
        xT[:, : counts[e]] = xb[idx[e]].T
        # [F1, cap] -> [128, K1, cap] SBUF tile layout.
        xT4 = xT.reshape(K1, 128, cap).transpose(1, 0, 2)

        w1p = np.zeros((128, M1, K1 * 128 + 2), BF16)
        w1p[:, :, : K1 * 128] = (
            W1b[e].reshape(K1, 128, M1, 128).transpose(1, 2, 0, 3)
            .reshape(128, M1, K1 * 128)
        )
        # b1 [F2] -> [128 partitions (f2-within), M1] fp32 bits.
        w1p[:, :, K1 * 128 :] = _pack_bias_cols(b1[e].reshape(M1, 128).T)

        w2p = np.zeros((128, K2 * 128 + 2), BF16)
        w2pad = np.zeros((128, K2, 128), BF16)
        w2pad[:, :, :C] = W2b[e].reshape(K2, 128, C).transpose(1, 0, 2)
        w2p[:, : K2 * 128] = w2pad.reshape(128, K2 * 128)
        w2p[:C, K2 * 128 :] = _pack_bias_cols(b2[e])

        m = {
            "w1": np.ascontiguousarray(w1p),
            "w2": np.ascontiguousarray(w2p),
        }
        n0 = 0
        for ci, cn in enumerate(chunks):
            m[f"xt{ci}"] = np.ascontiguousarray(xT4[:, :, n0 : n0 + cn])
            n0 += cn
        in_maps.append(m)

    res = run_bass_kernel_spmd(nc, in_maps, core_ids=list(range(N_CORES)))

    out = np.empty((B, C), np.float32)
    for e in range(E):
        out[idx[e]] = res.results[e]["outT"][:, : counts[e]].T.astype(np.float32)
    return out


# revision 13
# speedup vs baseline: 1.0326x; 1.0080x over previous
"""Trainium2 Bass kernel for MoE routing (2-layer expert MLP + softmax).

Strategy: expert-parallel across the 8 NeuronCores. The reference computes
all 8 experts for every sample and then gathers the one selected by
`domain`; mathematically only the selected expert's MLP matters per sample.
The host groups samples by expert, core e receives only the ~B/8 samples
routed to expert e (padded to a uniform per-core capacity so all cores run
the same SPMD program) plus expert e's weights. Each core runs a dense
2-layer MLP + softmax in a transposed layout:

    hT[f2, n]  = relu(W1[:, f2].T @ xT[:, n] + b1[f2])   (PE + ACT)
    lT[c, n]   = W2[:, c].T @ hT[:, n]                   (PE)
    expT       = exp(lT + b2)                            (ACT)
    sT[c, n]   = ones[C,C].T @ expT                      (PE partition sum)
    out[c, n]  = expT * (1 / sT)                         (DVE)

All matmul operands are bfloat16 (PSUM accumulation stays fp32).

v2 timeline notes (from the v1 NTFF trace, core with max count 2104):
- The framework preamble ends ~6.3us; nothing the kernel does can start
  earlier. The framework epilogue costs ~3.5us after the last store lands.
- The MM stream already ran at the measured roofline (N/2.4GHz + 2.5ns
  per matmul, LDWEIGHTS fully hidden at N>=256), so v2 attacks the edges:
  * v1's first real matmul waited until 13.3us for w1's first m-block,
    which sat on the scalar ring BEHIND the b1/b2 bias transfers - tiny
    but descriptor-dense (128 x 16B / 64 x 4B descriptors ~ 2.6us of DMA
    engine grind). v2 bakes both biases INTO the weight tensors as two
    trailing bf16 columns per block (fp32 bit pattern split across two
    bf16 slots, read back via AP.bitcast(f32) - bit-exact) so the weight
    stream starts immediately and there are no fine-grained transfers.
  * v1 chunked the batch [512 x 4, 56]; the 56-col tail chunk paid 37
    LDWEIGHTS-exposed matmuls (~100ns each vs the 25ns floor, ~2.8us
    wasted). v2 chunks [256, ~462 x 4]: every matmul's moving dim is
    >=256 so LDWEIGHTS stays hidden, and the small FIRST chunk needs only
    512KB of x before the real stream can start.
  * Ring ownership instead of halving every transfer: scalar (HWDGE,
    fastest starter) carries w1 m0 then x0's k5:8 tail then w1 m1..m3/w2
    interleaved between chunk-0 waves; sync (HWDGE) carries x0 k0:5 and
    the k5:8 share of every later chunk (+ output stores); gpsimd (SWDGE,
    ~2.2us descriptor-gen lag but ~285GB/s once streaming) starts on
    chunk 1's k0:5 share immediately and owns that share for all later
    chunks. Consumers only wait on DMAs emitted earlier on the same ring,
    so each chunk's triggers are emitted right before/after the waves
    that need them, exactly as in v1.
- Warmup: the HAM clock gate cannot reach K=8/8 before first-busy+3.4us,
  and cold matmuls still retire work at half rate, so the warmup burst is
  sized only to bridge engine-boot (~6.85us) to x0/w1m0-ready (~9.5us);
  real waves start cold and warm up in place.
- exp tiles are two persistent buffers zeroed once during the fill (rows
  C..127 must be zero for the ones-matmul partition sum); v1 re-memset
  them every chunk.
"""

import math
from collections import deque

import ml_dtypes
import numpy as np

import concourse.bacc as bacc
import concourse.bass as bass
import concourse.mybir as mybir
import concourse.tile as tile
from concourse.bass import ds
from concourse.bass_utils import run_bass_kernel_spmd

N_CORES = 8
BF16 = ml_dtypes.bfloat16

K_SPLIT = 5  # x k-blocks 0:5 on sync, 5:8 on scalar (~250GB/s aggregate)
WU_BIG = 8  # 512-col warmup matmuls (cold, ~427ns each: covers the 3.4us HAM window)
WU_SMALL = 24  # 128-col warmup matmuls (post-warm ~81ns each, bridge to data-ready)

_program_cache: dict[tuple, object] = {}


def _chunk_sizes(cap: int) -> list[int]:
    """[balanced <=512 chunks..., 256]: big chunks first (per-core DMA is
    only ~250-430GB/s aggregate, so the front must not need w1+x0+x1 all
    at once - a big chunk 0 buys the conveyor time), small chunk LAST so
    the post-stream tail (exp/norm/store) is short."""
    if cap <= 512:
        return [cap]
    cl = 256
    rest = cap - cl
    n = math.ceil(rest / 512)
    base, r = divmod(rest, n)
    return [base + (1 if i < r else 0) for i in range(n)] + [cl]


def _build_program(cap: int, F1: int, F2: int, C: int):
    key = (cap, F1, F2, C)
    if key in _program_cache:
        return _program_cache[key]

    assert F1 % 128 == 0 and F2 % 128 == 0
    K1 = F1 // 128
    M1 = F2 // 128
    K2 = F2 // 128
    assert C <= 128

    f32 = mybir.dt.float32
    bf16 = mybir.dt.bfloat16
    nc = bacc.Bacc(None, target_bir_lowering=False, debug=False)

    chunks = _chunk_sizes(cap)
    offs = [0]
    for cn in chunks:
        offs.append(offs[-1] + cn)
    n_chunks = len(chunks)

    x_d = [
        nc.dram_tensor(f"xt{ci}", [128, K1, cn], bf16, kind="ExternalInput")
        for ci, cn in enumerate(chunks)
    ]
    # w1: per m-block, K1 x 128 weight columns + 2 bf16 columns holding the
    # fp32 bias bit pattern (read via bitcast - exact).
    w1_d = nc.dram_tensor("w1", [128, M1, K1 * 128 + 2], bf16, kind="ExternalInput")
    # w2: K2 x 128 weight columns (output padded to 128: 64-wide matmuls
    # run ~1.5x slower) + 2 bias columns.
    w2_d = nc.dram_tensor("w2", [128, K2 * 128 + 2], bf16, kind="ExternalInput")
    out_d = nc.dram_tensor("outT", [C, cap], bf16, kind="ExternalOutput")

    with tile.TileContext(nc) as tc:
        with (
            tc.tile_pool(name="const", bufs=1) as const_pool,
            tc.tile_pool(name="expp", bufs=2) as e_pool,
            tc.tile_pool(name="xin", bufs=2) as x_pool,
            tc.tile_pool(name="h", bufs=2 * M1 + 2) as h_pool,
            tc.tile_pool(name="out", bufs=2) as o_pool,
            tc.tile_pool(name="rec", bufs=2) as r_pool,
            tc.tile_pool(name="ph", bufs=5, space="PSUM") as ph_pool,
            tc.tile_pool(name="pl", bufs=2, space="PSUM") as pl_pool,
            tc.tile_pool(name="pb", bufs=1, space="PSUM") as pb_pool,
        ):
            # Scalar ring (HWDGE): w1 m0 first - nothing descriptor-dense
            # ahead of it - then the x0 k5:8 tail. w1 m1..m3 and w2 are
            # emitted between chunk-0 waves below.
            w1_sb = const_pool.tile([128, M1, K1 * 128 + 2], bf16)
            nc.scalar.dma_start(w1_sb[:, 0, :], w1_d[:, 0, :])
            w2_sb = const_pool.tile([128, K2 * 128 + 2], bf16)

            # Sync ring (HWDGE, ~190GB/s but first data ~1.5us after
            # trigger): x0 k0:5. Scalar follows w1m0 with x0's k5:8 tail;
            # same-ring jobs serialize in order, so the two rings form two
            # parallel need-ordered conveyors with no cross-traffic.
            xt = []
            t = x_pool.tile([128, K1, chunks[0]], bf16, tag="xt", name="xt0")
            nc.sync.dma_start(t[:, :K_SPLIT, :], x_d[0][:, :K_SPLIT, :])
            nc.scalar.dma_start(t[:, K_SPLIT:, :], x_d[0][:, K_SPLIT:, :])
            xt.append(t)

            # GpSimd (SWDGE) does NO early DMA: its descriptor generation
            # interleaving with the HWDGE streams collapsed aggregate DMA
            # bandwidth to ~190GB/s in v2. It only carries x3/x4 k0:5
            # shares late (emitted after chunk 1/2's waves, gated by the
            # x-pool WAR) when the PE is busy and the fill is over.
            wu_x = const_pool.tile([128, 512], bf16)
            nc.gpsimd.memset(wu_x[:], 0.0)
            # ones[128, 128] all-ones stationary for the partition sum
            # (contraction and output dims padded to 128; exp rows C..127
            # are zero so the padding adds nothing).
            ones_cc = const_pool.tile([128, 128], bf16)
            nc.gpsimd.memset(ones_cc[:], 1.0)
            # Two persistent exp tiles, fully zeroed ONCE here; the EXP
            # activation only ever writes rows 0:C so rows C..127 stay 0.
            max_cn = max(chunks)
            exp_tiles = []
            for i in range(2):
                e = e_pool.tile([128, max_cn], bf16, tag="expt", name=f"exp{i}")
                nc.gpsimd.memset(e[:], 0.0)
                exp_tiles.append(e)

            # Warmup: bridge engine-boot (~6.85us) to first-data (~9.5us).
            # Cold matmuls retire real work at half rate, so undershoot
            # beats overshoot; HAM goes warm at first-busy+3.4us no matter
            # what we do here.
            for i in range(WU_BIG):
                wu_ps = ph_pool.tile([128, 512], f32, tag="ph", name=f"wu{i}")
                nc.tensor.matmul(
                    wu_ps[:], wu_x[:, :128], wu_x[:], start=True, stop=True
                )
            for i in range(WU_SMALL):
                wu_ps = ph_pool.tile([128, 128], f32, tag="ph", name=f"wv{i}")
                nc.tensor.matmul(
                    wu_ps[:], wu_x[:, :128], wu_x[:, :128], start=True, stop=True
                )

            b1_ap = [
                w1_sb[:, m, K1 * 128 : K1 * 128 + 2].bitcast(f32)
                for m in range(M1)
            ]
            b2_ap = w2_sb[0:C, K2 * 128 : K2 * 128 + 2].bitcast(f32)

            stages: deque = deque()

            def stage_l2(ci: int, cn: int, ht: list):
                pl = pl_pool.tile([128, cn], f32, tag="pl")
                for k in range(K2):
                    nc.tensor.matmul(
                        pl[:],
                        w2_sb[:, k * 128 : (k + 1) * 128],
                        ht[k][:],
                        start=(k == 0),
                        stop=(k == K2 - 1),
                    )
                expt = exp_tiles[ci % 2]
                nc.scalar.activation(
                    expt[0:C, :cn],
                    pl[0:C, :],
                    mybir.ActivationFunctionType.Exp,
                    bias=b2_ap,
                )
                stages.append(lambda: stage_norm(ci, cn, expt))

            def stage_norm(ci: int, cn: int, expt):
                pb = pb_pool.tile([128, cn], f32, tag="pb")
                nc.tensor.matmul(
                    pb[:], ones_cc[:], expt[:, :cn], start=True, stop=True
                )
                rec = r_pool.tile([C, cn], f32, tag="rec")
                nc.vector.reciprocal_approx_fast(rec[:], pb[0:C, :])
                ot = o_pool.tile([C, cn], bf16, tag="ot")
                nc.vector.tensor_mul(ot[:], expt[0:C, :cn], rec[:])
                # Mid-stream stores ride the otherwise-idle gpsimd ring so
                # the sync conveyor's x prefetches never queue behind a
                # store trigger (which stalls the engine until mul is
                # done). Only the LAST store - on the critical tail - uses
                # sync HWDGE (SWDGE descriptor generation costs ~1us).
                eng = nc.sync if ci == n_chunks - 1 else nc.gpsimd
                eng.dma_start(out_d[:, ds(offs[ci], cn)], ot[:])

            for ci, cn in enumerate(chunks):
                ht = []
                for m in range(M1):
                    ph = ph_pool.tile([128, cn], f32, tag="ph")
                    for k in range(K1):
                        nc.tensor.matmul(
                            ph[:],
                            w1_sb[:, m, k * 128 : (k + 1) * 128],
                            xt[ci][:, k, :],
                            start=(k == 0),
                            stop=(k == K1 - 1),
                        )
                    hm = h_pool.tile([128, cn], bf16, tag="ht")
                    nc.scalar.activation(
                        hm[:],
                        ph[:],
                        mybir.ActivationFunctionType.Relu,
                        bias=b1_ap[m],
                    )
                    ht.append(hm)
                    if ci == 0 and m + 1 < M1:
                        # Emitted AFTER wave m so wave m+1 (not wave m)
                        # carries the wait for this transfer.
                        nc.scalar.dma_start(
                            w1_sb[:, m + 1, :], w1_d[:, m + 1, :]
                        )
                    if ci == 0 and m == 2:
                        nc.scalar.dma_start(w2_sb[:], w2_d[:])
                    if stages:
                        stages.popleft()()
                # Next chunk's x, emitted AFTER this chunk's waves so this
                # chunk's matmuls don't wait on it (a consumer waits on
                # every DMA emitted earlier on the same ring). x1/x2 ride
                # the two HWDGE conveyors right behind the fill; x3/x4
                # move their k0:5 bulk to gpsimd (by then its SWDGE lag is
                # harmless and the HWDGE rings carry stores + k5:8).
                nxt = ci + 1
                if nxt < n_chunks:
                    cnn = chunks[nxt]
                    t = x_pool.tile(
                        [128, K1, cnn], bf16, tag="xt", name=f"xt{nxt}"
                    )
                    if nxt <= 2:
                        # x1/x2 ride both HWDGE conveyors behind the fill.
                        nc.sync.dma_start(
                            t[:, :K_SPLIT, :], x_d[nxt][:, :K_SPLIT, :]
                        )
                        nc.scalar.dma_start(
                            t[:, K_SPLIT:, :], x_d[nxt][:, K_SPLIT:, :]
                        )
                    else:
                        # By x3 the fill is long over; one sync job each,
                        # gated by the x-pool WAR to chunk nxt-2's end.
                        nc.sync.dma_start(t[:], x_d[nxt][:])
                    xt.append(t)
                stages.append(lambda ci=ci, cn=cn, ht=ht: stage_l2(ci, cn, ht))
            while stages:
                stages.popleft()()

    nc.compile()
    _program_cache[key] = nc
    return nc


def _pack_bias_cols(b: np.ndarray) -> np.ndarray:
    """fp32 [..., n] -> bf16 [..., n, 2] bit-pattern split."""
    a = np.ascontiguousarray(b, dtype="<f4")
    return a.view(np.uint16).view(BF16).reshape(*a.shape, 2)


def kernel(domain, x, W1, b1, W2, b2):
    domain = np.asarray(domain)
    x = np.ascontiguousarray(np.asarray(x, dtype=np.float32))
    W1 = np.asarray(W1, dtype=np.float32)
    b1 = np.asarray(b1, dtype=np.float32)
    W2 = np.asarray(W2, dtype=np.float32)
    b2 = np.asarray(b2, dtype=np.float32)

    B, F1 = x.shape
    E, _, F2 = W1.shape
    C = W2.shape[2]
    K1 = F1 // 128
    K2 = F2 // 128
    M1 = F2 // 128
    assert E == N_CORES

    xb = x.astype(BF16)
    W1b = W1.astype(BF16)
    W2b = W2.astype(BF16)

    idx = [np.nonzero(domain == e)[0] for e in range(E)]
    counts = [len(i) for i in idx]
    cap = max(512, max(counts))
    chunks = _chunk_sizes(cap)

    nc = _build_program(cap, F1, F2, C)

    in_maps = []
    for e in range(E):
        xT = np.zeros((F1, cap), BF16)
        xT[:, : counts[e]] = xb[idx[e]].T
        # [F1, cap] -> [128, K1, cap] SBUF tile layout.
        xT4 = xT.reshape(K1, 128, cap).transpose(1, 0, 2)

        w1p = np.zeros((128, M1, K1 * 128 + 2), BF16)
        w1p[:, :, : K1 * 128] = (
            W1b[e].reshape(K1, 128, M1, 128).transpose(1, 2, 0, 3)
            .reshape(128, M1, K1 * 128)
        )
        # b1 [F2] -> [128 partitions (f2-within), M1] fp32 bits.
        w1p[:, :, K1 * 128 :] = _pack_bias_cols(b1[e].reshape(M1, 128).T)

        w2p = np.zeros((128, K2 * 128 + 2), BF16)
        w2pad = np.zeros((128, K2, 128), BF16)
        w2pad[:, :, :C] = W2b[e].reshape(K2, 128, C).transpose(1, 0, 2)
        w2p[:, : K2 * 128] = w2pad.reshape(128, K2 * 128)
        w2p[:C, K2 * 128 :] = _pack_bias_cols(b2[e])

        m = {
            "w1": np.ascontiguousarray(w1p),
            "w2": np.ascontiguousarray(w2p),
        }
        n0 = 0
        for ci, cn in enumerate(chunks):
            m[f"xt{ci}"] = np.ascontiguousarray(xT4[:, :, n0 : n0 + cn])
            n0 += cn
        in_maps.append(m)

    res = run_bass_kernel_spmd(nc, in_maps, core_ids=list(range(N_CORES)))

    out = np.empty((B, C), np.float32)
    for e in range(E):
        out[idx[e]] = res.results[e]["outT"][:, : counts[e]].T.astype(np.float32)
    return out


# revision 17
# speedup vs baseline: 1.0870x; 1.0526x over previous
"""Trainium2 Bass kernel for MoE routing (2-layer expert MLP + softmax).

Strategy: expert-parallel across the 8 NeuronCores. The reference computes
all 8 experts for every sample and then gathers the one selected by
`domain`; mathematically only the selected expert's MLP matters per sample.
The host groups samples by expert, core e receives only the ~B/8 samples
routed to expert e (padded to a uniform per-core capacity so all cores run
the same SPMD program) plus expert e's weights. Each core runs a dense
2-layer MLP + softmax in a transposed layout:

    hT[f2, n]  = relu(W1[:, f2].T @ xT[:, n] + b1[f2])   (PE + ACT)
    lT[c, n]   = W2[:, c].T @ hT[:, n]                   (PE)
    expT       = exp(lT + b2)                            (ACT)
    sT[c, n]   = ones[C,C].T @ expT                      (PE partition sum)
    out[c, n]  = expT * (1 / sT)                         (DVE)

All matmul operands are bfloat16 (PSUM accumulation stays fp32).

v2 timeline notes (from the v1 NTFF trace, core with max count 2104):
- The framework preamble ends ~6.3us; nothing the kernel does can start
  earlier. The framework epilogue costs ~3.5us after the last store lands.
- The MM stream already ran at the measured roofline (N/2.4GHz + 2.5ns
  per matmul, LDWEIGHTS fully hidden at N>=256), so v2 attacks the edges:
  * v1's first real matmul waited until 13.3us for w1's first m-block,
    which sat on the scalar ring BEHIND the b1/b2 bias transfers - tiny
    but descriptor-dense (128 x 16B / 64 x 4B descriptors ~ 2.6us of DMA
    engine grind). v2 bakes both biases INTO the weight tensors as two
    trailing bf16 columns per block (fp32 bit pattern split across two
    bf16 slots, read back via AP.bitcast(f32) - bit-exact) so the weight
    stream starts immediately and there are no fine-grained transfers.
  * v1 chunked the batch [512 x 4, 56]; the 56-col tail chunk paid 37
    LDWEIGHTS-exposed matmuls (~100ns each vs the 25ns floor, ~2.8us
    wasted). v2 chunks [256, ~462 x 4]: every matmul's moving dim is
    >=256 so LDWEIGHTS stays hidden, and the small FIRST chunk needs only
    512KB of x before the real stream can start.
  * Ring ownership instead of halving every transfer: scalar (HWDGE,
    fastest starter) carries w1 m0 then x0's k5:8 tail then w1 m1..m3/w2
    interleaved between chunk-0 waves; sync (HWDGE) carries x0 k0:5 and
    the k5:8 share of every later chunk (+ output stores); gpsimd (SWDGE,
    ~2.2us descriptor-gen lag but ~285GB/s once streaming) starts on
    chunk 1's k0:5 share immediately and owns that share for all later
    chunks. Consumers only wait on DMAs emitted earlier on the same ring,
    so each chunk's triggers are emitted right before/after the waves
    that need them, exactly as in v1.
- Warmup: the HAM clock gate cannot reach K=8/8 before first-busy+3.4us,
  and cold matmuls still retire work at half rate, so the warmup burst is
  sized only to bridge engine-boot (~6.85us) to x0/w1m0-ready (~9.5us);
  real waves start cold and warm up in place.
- exp tiles are two persistent buffers zeroed once during the fill (rows
  C..127 must be zero for the ones-matmul partition sum); v1 re-memset
  them every chunk.
"""

import math
from collections import deque

import ml_dtypes
import numpy as np

import concourse.bacc as bacc
import concourse.bass as bass
import concourse.mybir as mybir
import concourse.tile as tile
from concourse.bass import ds
from concourse.bass_utils import run_bass_kernel_spmd

N_CORES = 8
BF16 = ml_dtypes.bfloat16

K_SPLIT = 4  # x k-blocks halved across the sync and gpsimd rings
WU_BIG = 8  # 512-col warmup matmuls (cold, ~427ns each: covers the 3.4us HAM window)
WU_SMALL = 34  # 128-col warmup matmuls (post-warm ~60-80ns each, bridge to x0-ready ~13us)

_program_cache: dict[tuple, object] = {}


def _chunk_sizes(cap: int) -> list[int]:
    """[balanced <=512 chunks..., 256]: big chunks first (per-core DMA is
    only ~250-430GB/s aggregate, so the front must not need w1+x0+x1 all
    at once - a big chunk 0 buys the conveyor time), small chunk LAST so
    the post-stream tail (exp/norm/store) is short."""
    if cap <= 512:
        return [cap]
    cl = 256
    rest = cap - cl
    n = math.ceil(rest / 512)
    base, r = divmod(rest, n)
    return [base + (1 if i < r else 0) for i in range(n)] + [cl]


def _build_program(cap: int, F1: int, F2: int, C: int):
    key = (cap, F1, F2, C)
    if key in _program_cache:
        return _program_cache[key]

    assert F1 % 128 == 0 and F2 % 128 == 0
    K1 = F1 // 128
    M1 = F2 // 128
    K2 = F2 // 128
    assert C <= 128

    f32 = mybir.dt.float32
    bf16 = mybir.dt.bfloat16
    nc = bacc.Bacc(None, target_bir_lowering=False, debug=False)

    chunks = _chunk_sizes(cap)
    offs = [0]
    for cn in chunks:
        offs.append(offs[-1] + cn)
    n_chunks = len(chunks)

    x_d = [
        nc.dram_tensor(f"xt{ci}", [128, K1, cn], bf16, kind="ExternalInput")
        for ci, cn in enumerate(chunks)
    ]
    # w1: per m-block, K1 x 128 weight columns + 2 bf16 columns holding the
    # fp32 bias bit pattern (read via bitcast - exact).
    w1_d = nc.dram_tensor("w1", [128, M1, K1 * 128 + 2], bf16, kind="ExternalInput")
    # w2: K2 x 128 weight columns (output padded to 128: 64-wide matmuls
    # run ~1.5x slower) + 2 bias columns.
    w2_d = nc.dram_tensor("w2", [128, K2 * 128 + 2], bf16, kind="ExternalInput")
    out_d = nc.dram_tensor("outT", [C, cap], bf16, kind="ExternalOutput")

    with tile.TileContext(nc) as tc:
        with (
            tc.tile_pool(name="const", bufs=1) as const_pool,
            tc.tile_pool(name="expp", bufs=2) as e_pool,
            tc.tile_pool(name="xin", bufs=2) as x_pool,
            tc.tile_pool(name="h", bufs=2 * M1 + 2) as h_pool,
            tc.tile_pool(name="out", bufs=2) as o_pool,
            tc.tile_pool(name="rec", bufs=2) as r_pool,
            tc.tile_pool(name="ph", bufs=5, space="PSUM") as ph_pool,
            tc.tile_pool(name="pl", bufs=2, space="PSUM") as pl_pool,
            tc.tile_pool(name="pb", bufs=1, space="PSUM") as pb_pool,
        ):
            # Scalar ring (HWDGE): w1 m0 first - nothing descriptor-dense
            # ahead of it - then the x0 k5:8 tail. w1 m1..m3 and w2 are
            # emitted between chunk-0 waves below.
            w1_sb = const_pool.tile([128, M1, K1 * 128 + 2], bf16)
            nc.scalar.dma_start(w1_sb[:, 0, :], w1_d[:, 0, :])
            w2_sb = const_pool.tile([128, K2 * 128 + 2], bf16)

            # Warmup operand memset FIRST on the gpsimd queue (before its
            # DMA half) so the warmup starts as soon as the engine boots.
            wu_x = const_pool.tile([128, 512], bf16)
            nc.gpsimd.memset(wu_x[:], 0.0)

            # x0 halves on the sync + gpsimd rings (v1-proven pacing: the
            # per-core DMA aggregate is only ~210GB/s, so the scalar ring
            # keeps the whole w1/w2 stream to itself while x rides the
            # other two rings, one chunk in flight at a time).
            xt = []
            t = x_pool.tile([128, K1, chunks[0]], bf16, tag="xt", name="xt0")
            nc.sync.dma_start(t[:, :K_SPLIT, :], x_d[0][:, :K_SPLIT, :])
            nc.gpsimd.dma_start(t[:, K_SPLIT:, :], x_d[0][:, K_SPLIT:, :])
            xt.append(t)

            # ones[128, 128] all-ones stationary for the partition sum
            # (contraction and output dims padded to 128; exp rows C..127
            # are zero so the padding adds nothing).
            ones_cc = const_pool.tile([128, 128], bf16)
            nc.gpsimd.memset(ones_cc[:], 1.0)
            # Two persistent exp tiles, fully zeroed ONCE here; the EXP
            # activation only ever writes rows 0:C so rows C..127 stay 0.
            max_cn = max(chunks)
            exp_tiles = []
            for i in range(2):
                e = e_pool.tile([128, max_cn], bf16, tag="expt", name=f"exp{i}")
                nc.gpsimd.memset(e[:], 0.0)
                exp_tiles.append(e)

            # Warmup: bridge engine-boot (~6.85us) to first-data (~9.5us).
            # Cold matmuls retire real work at half rate, so undershoot
            # beats overshoot; HAM goes warm at first-busy+3.4us no matter
            # what we do here.
            for i in range(WU_BIG):
                wu_ps = ph_pool.tile([128, 512], f32, tag="ph", name=f"wu{i}")
                nc.tensor.matmul(
                    wu_ps[:], wu_x[:, :128], wu_x[:], start=True, stop=True
                )
            for i in range(WU_SMALL):
                wu_ps = ph_pool.tile([128, 128], f32, tag="ph", name=f"wv{i}")
                nc.tensor.matmul(
                    wu_ps[:], wu_x[:, :128], wu_x[:, :128], start=True, stop=True
                )

            b1_ap = [
                w1_sb[:, m, K1 * 128 : K1 * 128 + 2].bitcast(f32)
                for m in range(M1)
            ]
            b2_ap = w2_sb[0:C, K2 * 128 : K2 * 128 + 2].bitcast(f32)

            stages: deque = deque()

            def stage_l2(ci: int, cn: int, ht: list):
                pl = pl_pool.tile([128, cn], f32, tag="pl")
                for k in range(K2):
                    nc.tensor.matmul(
                        pl[:],
                        w2_sb[:, k * 128 : (k + 1) * 128],
                        ht[k][:],
                        start=(k == 0),
                        stop=(k == K2 - 1),
                    )
                expt = exp_tiles[ci % 2]
                nc.scalar.activation(
                    expt[0:C, :cn],
                    pl[0:C, :],
                    mybir.ActivationFunctionType.Exp,
                    bias=b2_ap,
                )
                stages.append(lambda: stage_norm(ci, cn, expt))

            def stage_norm(ci: int, cn: int, expt):
                pb = pb_pool.tile([128, cn], f32, tag="pb")
                nc.tensor.matmul(
                    pb[:], ones_cc[:], expt[:, :cn], start=True, stop=True
                )
                rec = r_pool.tile([C, cn], f32, tag="rec")
                nc.vector.reciprocal_approx_fast(rec[:], pb[0:C, :])
                ot = o_pool.tile([C, cn], bf16, tag="ot")
                nc.vector.tensor_mul(ot[:], expt[0:C, :cn], rec[:])
                # Sync HWDGE, not gpsimd SWDGE: SWDGE descriptor
                # generation costs ~1us and the final store sits on the
                # critical tail.
                nc.sync.dma_start(out_d[:, ds(offs[ci], cn)], ot[:])

            for ci, cn in enumerate(chunks):
                ht = []
                for m in range(M1):
                    ph = ph_pool.tile([128, cn], f32, tag="ph")
                    for k in range(K1):
                        nc.tensor.matmul(
                            ph[:],
                            w1_sb[:, m, k * 128 : (k + 1) * 128],
                            xt[ci][:, k, :],
                            start=(k == 0),
                            stop=(k == K1 - 1),
                        )
                    hm = h_pool.tile([128, cn], bf16, tag="ht")
                    nc.scalar.activation(
                        hm[:],
                        ph[:],
                        mybir.ActivationFunctionType.Relu,
                        bias=b1_ap[m],
                    )
                    ht.append(hm)
                    if ci == 0 and m + 1 < M1:
                        # Emitted AFTER wave m so wave m+1 (not wave m)
                        # carries the wait for this transfer.
                        nc.scalar.dma_start(
                            w1_sb[:, m + 1, :], w1_d[:, m + 1, :]
                        )
                    if ci == 0 and m == 2:
                        nc.scalar.dma_start(w2_sb[:], w2_d[:])
                    if stages:
                        stages.popleft()()
                # Next chunk's x, emitted AFTER this chunk's waves so this
                # chunk's matmuls don't wait on it (a consumer waits on
                # every DMA emitted earlier on the same ring). x1/x2 ride
                # the two HWDGE conveyors right behind the fill; x3/x4
                # move their k0:5 bulk to gpsimd (by then its SWDGE lag is
                # harmless and the HWDGE rings carry stores + k5:8).
                # Prefetch next chunk's x AFTER this chunk's waves: a
                # consumer waits on every DMA emitted earlier on the same
                # queue, so emitting the prefetch first would make THIS
                # chunk's matmuls wait for the NEXT chunk's transfer.
                nxt = ci + 1
                if nxt < n_chunks:
                    cnn = chunks[nxt]
                    t = x_pool.tile(
                        [128, K1, cnn], bf16, tag="xt", name=f"xt{nxt}"
                    )
                    # Gate the prefetch's ISSUE (not just its consumers)
                    # behind the current chunk's data: this corner copy
                    # reads chunk ci (both DMA halves) and writes a corner
                    # of chunk ci+1's tile, so the WAW dependency stops the
                    # next D2D from interleaving its descriptors with the
                    # in-flight transfer and halving its bandwidth. The DMA
                    # overwrites the corner immediately.
                    nc.vector.tensor_copy(
                        t[0:1, :, 0:1], xt[ci][0:1, :, 0:1]
                    )
                    nc.sync.dma_start(
                        t[:, :K_SPLIT, :], x_d[nxt][:, :K_SPLIT, :]
                    )
                    nc.gpsimd.dma_start(
                        t[:, K_SPLIT:, :], x_d[nxt][:, K_SPLIT:, :]
                    )
                    xt.append(t)
                stages.append(lambda ci=ci, cn=cn, ht=ht: stage_l2(ci, cn, ht))
            while stages:
                stages.popleft()()

    nc.compile()
    _program_cache[key] = nc
    return nc


def _pack_bias_cols(b: np.ndarray) -> np.ndarray:
    """fp32 [..., n] -> bf16 [..., n, 2] bit-pattern split."""
    a = np.ascontiguousarray(b, dtype="<f4")
    return a.view(np.uint16).view(BF16).reshape(*a.shape, 2)


def kernel(domain, x, W1, b1, W2, b2):
    domain = np.asarray(domain)
    x = np.ascontiguousarray(np.asarray(x, dtype=np.float32))
    W1 = np.asarray(W1, dtype=np.float32)
    b1 = np.asarray(b1, dtype=np.float32)
    W2 = np.asarray(W2, dtype=np.float32)
    b2 = np.asarray(b2, dtype=np.float32)

    B, F1 = x.shape
    E, _, F2 = W1.shape
    C = W2.shape[2]
    K1 = F1 // 128
    K2 = F2 // 128
    M1 = F2 // 128
    assert E == N_CORES

    xb = x.astype(BF16)
    W1b = W1.astype(BF16)
    W2b = W2.astype(BF16)

    idx = [np.nonzero(domain == e)[0] for e in range(E)]
    counts = [len(i) for i in idx]
    cap = max(512, max(counts))
    chunks = _chunk_sizes(cap)

    nc = _build_program(cap, F1, F2, C)

    in_maps = []
    for e in range(E):
        xT = np.zeros((F1, cap), BF16)
        xT[:, : counts[e]] = xb[idx[e]].T
        # [F1, cap] -> [128, K1, cap] SBUF tile layout.
        xT4 = xT.reshape(K1, 128, cap).transpose(1, 0, 2)

        w1p = np.zeros((128, M1, K1 * 128 + 2), BF16)
        w1p[:, :, : K1 * 128] = (
            W1b[e].reshape(K1, 128, M1, 128).transpose(1, 2, 0, 3)
            .reshape(128, M1, K1 * 128)
        )
        # b1 [F2] -> [128 partitions (f2-within), M1] fp32 bits.
        w1p[:, :, K1 * 128 :] = _pack_bias_cols(b1[e].reshape(M1, 128).T)

        w2p = np.zeros((128, K2 * 128 + 2), BF16)
        w2pad = np.zeros((128, K2, 128), BF16)
        w2pad[:, :, :C] = W2b[e].reshape(K2, 128, C).transpose(1, 0, 2)
        w2p[:, : K2 * 128] = w2pad.reshape(128, K2 * 128)
        w2p[:C, K2 * 128 :] = _pack_bias_cols(b2[e])

        m = {
            "w1": np.ascontiguousarray(w1p),
            "w2": np.ascontiguousarray(w2p),
        }
        n0 = 0
        for ci, cn in enumerate(chunks):
            m[f"xt{ci}"] = np.ascontiguousarray(xT4[:, :, n0 : n0 + cn])
            n0 += cn
        in_maps.append(m)

    res = run_bass_kernel_spmd(nc, in_maps, core_ids=list(range(N_CORES)))

    out = np.empty((B, C), np.float32)
    for e in range(E):
        out[idx[e]] = res.results[e]["outT"][:, : counts[e]].T.astype(np.float32)
    return out


# revision 25
# speedup vs baseline: 1.1279x; 1.0377x over previous
"""Trainium2 Bass kernel for MoE routing (2-layer expert MLP + softmax).

Strategy: expert-parallel across the 8 NeuronCores. The reference computes
all 8 experts for every sample and then gathers the one selected by
`domain`; mathematically only the selected expert's MLP matters per sample.
The host groups samples by expert, core e receives only the ~B/8 samples
routed to expert e (padded to a uniform per-core capacity so all cores run
the same SPMD program) plus expert e's weights. Each core runs a dense
2-layer MLP + softmax in a transposed layout:

    hT[f2, n]  = relu(W1[:, f2].T @ xT[:, n] + b1[f2])   (PE + ACT)
    lT[c, n]   = W2[:, c].T @ hT[:, n]                   (PE)
    expT       = exp(lT + b2)                            (ACT)
    sT[c, n]   = ones[C,C].T @ expT                      (PE partition sum)
    out[c, n]  = expT * (1 / sT)                         (DVE)

All matmul operands are bfloat16 (PSUM accumulation stays fp32).

v2 timeline notes (from the v1 NTFF trace, core with max count 2104):
- The framework preamble ends ~6.3us; nothing the kernel does can start
  earlier. The framework epilogue costs ~3.5us after the last store lands.
- The MM stream already ran at the measured roofline (N/2.4GHz + 2.5ns
  per matmul, LDWEIGHTS fully hidden at N>=256), so v2 attacks the edges:
  * v1's first real matmul waited until 13.3us for w1's first m-block,
    which sat on the scalar ring BEHIND the b1/b2 bias transfers - tiny
    but descriptor-dense (128 x 16B / 64 x 4B descriptors ~ 2.6us of DMA
    engine grind). v2 bakes both biases INTO the weight tensors as two
    trailing bf16 columns per block (fp32 bit pattern split across two
    bf16 slots, read back via AP.bitcast(f32) - bit-exact) so the weight
    stream starts immediately and there are no fine-grained transfers.
  * v1 chunked the batch [512 x 4, 56]; the 56-col tail chunk paid 37
    LDWEIGHTS-exposed matmuls (~100ns each vs the 25ns floor, ~2.8us
    wasted). v2 chunks [256, ~462 x 4]: every matmul's moving dim is
    >=256 so LDWEIGHTS stays hidden, and the small FIRST chunk needs only
    512KB of x before the real stream can start.
  * Ring ownership instead of halving every transfer: scalar (HWDGE,
    fastest starter) carries w1 m0 then x0's k5:8 tail then w1 m1..m3/w2
    interleaved between chunk-0 waves; sync (HWDGE) carries x0 k0:5 and
    the k5:8 share of every later chunk (+ output stores); gpsimd (SWDGE,
    ~2.2us descriptor-gen lag but ~285GB/s once streaming) starts on
    chunk 1's k0:5 share immediately and owns that share for all later
    chunks. Consumers only wait on DMAs emitted earlier on the same ring,
    so each chunk's triggers are emitted right before/after the waves
    that need them, exactly as in v1.
- Warmup: the HAM clock gate cannot reach K=8/8 before first-busy+3.4us,
  and cold matmuls still retire work at half rate, so the warmup burst is
  sized only to bridge engine-boot (~6.85us) to x0/w1m0-ready (~9.5us);
  real waves start cold and warm up in place.
- exp tiles are two persistent buffers zeroed once during the fill (rows
  C..127 must be zero for the ones-matmul partition sum); v1 re-memset
  them every chunk.
"""

import math
from collections import deque

import ml_dtypes
import numpy as np

import concourse.bacc as bacc
import concourse.bass as bass
import concourse.mybir as mybir
import concourse.tile as tile
from concourse.bass import ds
from concourse.bass_utils import run_bass_kernel_spmd

N_CORES = 8
BF16 = ml_dtypes.bfloat16

K_SPLIT = 4  # x k-blocks halved across the sync and gpsimd rings
WU_BIG = 8  # 512-col warmup matmuls (cold, ~427ns each: covers the 3.4us HAM window)
WU_SMALL = 34  # 128-col warmup matmuls (post-warm ~60-80ns each, bridge to x0-ready ~13us)

_program_cache: dict[tuple, object] = {}


def _chunk_sizes(cap: int) -> list[int]:
    """[balanced <=512 chunks..., 256]: big chunks first (per-core DMA is
    only ~250-430GB/s aggregate, so the front must not need w1+x0+x1 all
    at once - a big chunk 0 buys the conveyor time), small chunk LAST so
    the post-stream tail (exp/norm/store) is short."""
    if cap <= 512:
        return [cap]
    cl = 256
    rest = cap - cl
    n = math.ceil(rest / 512)
    base, r = divmod(rest, n)
    return [base + (1 if i < r else 0) for i in range(n)] + [cl]


def _build_program(cap: int, F1: int, F2: int, C: int):
    key = (cap, F1, F2, C)
    if key in _program_cache:
        return _program_cache[key]

    assert F1 % 128 == 0 and F2 % 128 == 0
    K1 = F1 // 128
    M1 = F2 // 128
    K2 = F2 // 128
    assert C <= 128

    f32 = mybir.dt.float32
    bf16 = mybir.dt.bfloat16
    nc = bacc.Bacc(None, target_bir_lowering=False, debug=False)

    chunks = _chunk_sizes(cap)
    offs = [0]
    for cn in chunks:
        offs.append(offs[-1] + cn)
    n_chunks = len(chunks)

    x_d = [
        nc.dram_tensor(f"xt{ci}", [128, K1, cn], bf16, kind="ExternalInput")
        for ci, cn in enumerate(chunks)
    ]
    # w1: per m-block, K1 x 128 weight columns + 2 bf16 columns holding the
    # fp32 bias bit pattern (read via bitcast - exact).
    w1_d = nc.dram_tensor("w1", [128, M1, K1 * 128 + 2], bf16, kind="ExternalInput")
    # w2: K2 x C(=64) real weight columns + 2 bias columns. Layer 2 runs
    # as PAIRS of concurrent 64-wide matmuls in the two PE column-group
    # halves (tile_position (0,0) / (0,64)), so no 128-padding is needed;
    # a DVE add folds psum[0:64] + psum[64:128] into the logits.
    assert C == 64 and K2 % 2 == 0
    w2_d = nc.dram_tensor("w2", [128, K2 * C + 2], bf16, kind="ExternalInput")
    # Unnormalized exp(logits); the softmax divide happens on the host
    # (exact fp32, replacing the device's approx-reciprocal) - this drops
    # the partition-sum matmul, reciprocal and multiply from the kernel.
    out_d = nc.dram_tensor("outT", [C, cap], bf16, kind="ExternalOutput")

    with tile.TileContext(nc) as tc:
        with (
            tc.tile_pool(name="const", bufs=1) as const_pool,
            tc.tile_pool(name="expp", bufs=2) as e_pool,
            tc.tile_pool(name="xin", bufs=2) as x_pool,
            tc.tile_pool(name="h", bufs=2 * M1 + 2) as h_pool,
            tc.tile_pool(name="lg", bufs=2) as l_pool,
            tc.tile_pool(name="ph", bufs=5, space="PSUM") as ph_pool,
            tc.tile_pool(name="pl", bufs=2, space="PSUM") as pl_pool,
        ):
            # Scalar ring (HWDGE): w1 m0 first - nothing descriptor-dense
            # ahead of it - then the x0 k5:8 tail. w1 m1..m3 and w2 are
            # emitted between chunk-0 waves below.
            w1_sb = const_pool.tile([128, M1, K1 * 128 + 2], bf16)
            nc.scalar.dma_start(w1_sb[:, 0, :], w1_d[:, 0, :])
            w2_sb = const_pool.tile([128, K2 * C + 2], bf16)

            # Warmup operand memset FIRST on the gpsimd queue (before its
            # DMA half) so the warmup starts as soon as the engine boots.
            wu_x = const_pool.tile([128, 512], bf16)
            nc.gpsimd.memset(wu_x[:], 0.0)

            # x0 halves on the sync + gpsimd rings (v1-proven pacing: the
            # per-core DMA aggregate is only ~210GB/s, so the scalar ring
            # keeps the whole w1/w2 stream to itself while x rides the
            # other two rings, one chunk in flight at a time).
            xt = []
            t = x_pool.tile([128, K1, chunks[0]], bf16, tag="xt", name="xt0")
            nc.sync.dma_start(t[:, :K_SPLIT, :], x_d[0][:, :K_SPLIT, :])
            nc.gpsimd.dma_start(t[:, K_SPLIT:, :], x_d[0][:, K_SPLIT:, :])
            xt.append(t)



            # Warmup: bridge engine-boot (~6.85us) to first-data (~9.5us).
            # Cold matmuls retire real work at half rate, so undershoot
            # beats overshoot; HAM goes warm at first-busy+3.4us no matter
            # what we do here.
            for i in range(WU_BIG):
                wu_ps = ph_pool.tile([128, 512], f32, tag="ph", name=f"wu{i}")
                nc.tensor.matmul(
                    wu_ps[:], wu_x[:, :128], wu_x[:], start=True, stop=True
                )
            for i in range(WU_SMALL):
                wu_ps = ph_pool.tile([128, 128], f32, tag="ph", name=f"wv{i}")
                nc.tensor.matmul(
                    wu_ps[:], wu_x[:, :128], wu_x[:, :128], start=True, stop=True
                )

            b1_ap = [
                w1_sb[:, m, K1 * 128 : K1 * 128 + 2].bitcast(f32)
                for m in range(M1)
            ]
            b2_ap = w2_sb[0:C, K2 * C : K2 * C + 2].bitcast(f32)

            stages: deque = deque()

            def stage_l2(ci: int, cn: int, ht: list):
                pl = pl_pool.tile([128, cn], f32, tag="pl")
                for kp in range(K2 // 2):
                    ka, kb = 2 * kp, 2 * kp + 1
                    nc.tensor.matmul(
                        pl[0:C, :],
                        w2_sb[:, ka * C : (ka + 1) * C],
                        ht[ka][:],
                        start=(kp == 0),
                        stop=(kp == K2 // 2 - 1),
                        tile_position=(0, 0),
                    )
                    nc.tensor.matmul(
                        pl[C : 2 * C, :],
                        w2_sb[:, kb * C : (kb + 1) * C],
                        ht[kb][:],
                        start=(kp == 0),
                        stop=(kp == K2 // 2 - 1),
                        tile_position=(0, C),
                    )
                # DVE may read only one PSUM operand per instruction:
                # copy one half out, then add in place.
                lg = l_pool.tile([C, cn], f32, tag="lg")
                nc.vector.tensor_copy(lg[:], pl[C : 2 * C, :])
                nc.vector.tensor_add(lg[:], lg[:], pl[0:C, :])
                expt = e_pool.tile([C, cn], bf16, tag="expt")
                nc.scalar.activation(
                    expt[:],
                    lg[:],
                    mybir.ActivationFunctionType.Exp,
                    bias=b2_ap,
                )
                # Sync HWDGE, not gpsimd SWDGE: SWDGE descriptor
                # generation costs ~1us and the final store sits on the
                # critical tail.
                nc.sync.dma_start(out_d[:, ds(offs[ci], cn)], expt[:])

            for ci, cn in enumerate(chunks):
                ht = []
                for m in range(M1):
                    ph = ph_pool.tile([128, cn], f32, tag="ph")
                    for k in range(K1):
                        nc.tensor.matmul(
                            ph[:],
                            w1_sb[:, m, k * 128 : (k + 1) * 128],
                            xt[ci][:, k, :],
                            start=(k == 0),
                            stop=(k == K1 - 1),
                        )
                    hm = h_pool.tile([128, cn], bf16, tag="ht")
                    nc.scalar.activation(
                        hm[:],
                        ph[:],
                        mybir.ActivationFunctionType.Relu,
                        bias=b1_ap[m],
                    )
                    ht.append(hm)
                    if ci == 0 and m + 1 < M1:
                        # Emitted AFTER wave m so wave m+1 (not wave m)
                        # carries the wait for this transfer.
                        nc.scalar.dma_start(
                            w1_sb[:, m + 1, :], w1_d[:, m + 1, :]
                        )
                    if ci == 0 and m == 2:
                        nc.scalar.dma_start(w2_sb[:], w2_d[:])
                    if stages:
                        stages.popleft()()
                # Next chunk's x, emitted AFTER this chunk's waves so this
                # chunk's matmuls don't wait on it (a consumer waits on
                # every DMA emitted earlier on the same ring). x1/x2 ride
                # the two HWDGE conveyors right behind the fill; x3/x4
                # move their k0:5 bulk to gpsimd (by then its SWDGE lag is
                # harmless and the HWDGE rings carry stores + k5:8).
                # Prefetch next chunk's x AFTER this chunk's waves: a
                # consumer waits on every DMA emitted earlier on the same
                # queue, so emitting the prefetch first would make THIS
                # chunk's matmuls wait for the NEXT chunk's transfer.
                nxt = ci + 1
                if nxt < n_chunks:
                    cnn = chunks[nxt]
                    t = x_pool.tile(
                        [128, K1, cnn], bf16, tag="xt", name=f"xt{nxt}"
                    )
                    # Gate the prefetch's ISSUE (not just its consumers)
                    # behind the current chunk's data: this corner copy
                    # reads chunk ci (both DMA halves) and writes a corner
                    # of chunk ci+1's tile, so the WAW dependency stops the
                    # next D2D from interleaving its descriptors with the
                    # in-flight transfer and halving its bandwidth. The DMA
                    # overwrites the corner immediately.
                    nc.vector.tensor_copy(
                        t[0:1, :, 0:1], xt[ci][0:1, :, 0:1]
                    )
                    nc.sync.dma_start(
                        t[:, :K_SPLIT, :], x_d[nxt][:, :K_SPLIT, :]
                    )
                    nc.gpsimd.dma_start(
                        t[:, K_SPLIT:, :], x_d[nxt][:, K_SPLIT:, :]
                    )
                    xt.append(t)
                stages.append(lambda ci=ci, cn=cn, ht=ht: stage_l2(ci, cn, ht))
            while stages:
                stages.popleft()()

    nc.compile()
    _program_cache[key] = nc
    return nc


def _pack_bias_cols(b: np.ndarray) -> np.ndarray:
    """fp32 [..., n] -> bf16 [..., n, 2] bit-pattern split."""
    a = np.ascontiguousarray(b, dtype="<f4")
    return a.view(np.uint16).view(BF16).reshape(*a.shape, 2)


def kernel(domain, x, W1, b1, W2, b2):
    domain = np.asarray(domain)
    x = np.ascontiguousarray(np.asarray(x, dtype=np.float32))
    W1 = np.asarray(W1, dtype=np.float32)
    b1 = np.asarray(b1, dtype=np.float32)
    W2 = np.asarray(W2, dtype=np.float32)
    b2 = np.asarray(b2, dtype=np.float32)

    B, F1 = x.shape
    E, _, F2 = W1.shape
    C = W2.shape[2]
    K1 = F1 // 128
    K2 = F2 // 128
    M1 = F2 // 128
    assert E == N_CORES

    xb = x.astype(BF16)
    W1b = W1.astype(BF16)
    W2b = W2.astype(BF16)

    idx = [np.nonzero(domain == e)[0] for e in range(E)]
    counts = [len(i) for i in idx]
    cap = max(512, max(counts))
    chunks = _chunk_sizes(cap)

    nc = _build_program(cap, F1, F2, C)

    in_maps = []
    for e in range(E):
        xT = np.zeros((F1, cap), BF16)
        xT[:, : counts[e]] = xb[idx[e]].T
        # [F1, cap] -> [128, K1, cap] SBUF tile layout.
        xT4 = xT.reshape(K1, 128, cap).transpose(1, 0, 2)

        w1p = np.zeros((128, M1, K1 * 128 + 2), BF16)
        w1p[:, :, : K1 * 128] = (
            W1b[e].reshape(K1, 128, M1, 128).transpose(1, 2, 0, 3)
            .reshape(128, M1, K1 * 128)
        )
        # b1 [F2] -> [128 partitions (f2-within), M1] fp32 bits.
        w1p[:, :, K1 * 128 :] = _pack_bias_cols(b1[e].reshape(M1, 128).T)

        w2p = np.zeros((128, K2 * C + 2), BF16)
        w2p[:, : K2 * C] = (
            W2b[e].reshape(K2, 128, C).transpose(1, 0, 2).reshape(128, K2 * C)
        )
        w2p[:C, K2 * C :] = _pack_bias_cols(b2[e])

        m = {
            "w1": np.ascontiguousarray(w1p),
            "w2": np.ascontiguousarray(w2p),
        }
        n0 = 0
        for ci, cn in enumerate(chunks):
            m[f"xt{ci}"] = np.ascontiguousarray(xT4[:, :, n0 : n0 + cn])
            n0 += cn
        in_maps.append(m)

    res = run_bass_kernel_spmd(nc, in_maps, core_ids=list(range(N_CORES)))

    out = np.empty((B, C), np.float32)
    for e in range(E):
        # Device returns unnormalized exp(logits); exact softmax divide
        # here (replaces the device-side approx-reciprocal).
        et = res.results[e]["outT"][:, : counts[e]].T.astype(np.float32)
        out[idx[e]] = et / et.sum(axis=1, keepdims=True)
    return out


# revision 32
# speedup vs baseline: 1.1422x; 1.0126x over previous
"""Trainium2 Bass kernel for MoE routing (2-layer expert MLP + softmax).

Strategy: expert-parallel across the 8 NeuronCores. The reference computes
all 8 experts for every sample and then gathers the one selected by
`domain`; mathematically only the selected expert's MLP matters per sample.
The host groups samples by expert, core e receives only the ~B/8 samples
routed to expert e (padded to a uniform per-core capacity so all cores run
the same SPMD program) plus expert e's weights. Each core runs a dense
2-layer MLP + softmax in a transposed layout:

    hT[f2, n]  = relu(W1[:, f2].T @ xT[:, n] + b1[f2])   (PE + ACT)
    lT[c, n]   = W2[:, c].T @ hT[:, n]                   (PE)
    expT       = exp(lT + b2)                            (ACT)
    sT[c, n]   = ones[C,C].T @ expT                      (PE partition sum)
    out[c, n]  = expT * (1 / sT)                         (DVE)

All matmul operands are bfloat16 (PSUM accumulation stays fp32).

v2 timeline notes (from the v1 NTFF trace, core with max count 2104):
- The framework preamble ends ~6.3us; nothing the kernel does can start
  earlier. The framework epilogue costs ~3.5us after the last store lands.
- The MM stream already ran at the measured roofline (N/2.4GHz + 2.5ns
  per matmul, LDWEIGHTS fully hidden at N>=256), so v2 attacks the edges:
  * v1's first real matmul waited until 13.3us for w1's first m-block,
    which sat on the scalar ring BEHIND the b1/b2 bias transfers - tiny
    but descriptor-dense (128 x 16B / 64 x 4B descriptors ~ 2.6us of DMA
    engine grind). v2 bakes both biases INTO the weight tensors as two
    trailing bf16 columns per block (fp32 bit pattern split across two
    bf16 slots, read back via AP.bitcast(f32) - bit-exact) so the weight
    stream starts immediately and there are no fine-grained transfers.
  * v1 chunked the batch [512 x 4, 56]; the 56-col tail chunk paid 37
    LDWEIGHTS-exposed matmuls (~100ns each vs the 25ns floor, ~2.8us
    wasted). v2 chunks [256, ~462 x 4]: every matmul's moving dim is
    >=256 so LDWEIGHTS stays hidden, and the small FIRST chunk needs only
    512KB of x before the real stream can start.
  * Ring ownership instead of halving every transfer: scalar (HWDGE,
    fastest starter) carries w1 m0 then x0's k5:8 tail then w1 m1..m3/w2
    interleaved between chunk-0 waves; sync (HWDGE) carries x0 k0:5 and
    the k5:8 share of every later chunk (+ output stores); gpsimd (SWDGE,
    ~2.2us descriptor-gen lag but ~285GB/s once streaming) starts on
    chunk 1's k0:5 share immediately and owns that share for all later
    chunks. Consumers only wait on DMAs emitted earlier on the same ring,
    so each chunk's triggers are emitted right before/after the waves
    that need them, exactly as in v1.
- Warmup: the HAM clock gate cannot reach K=8/8 before first-busy+3.4us,
  and cold matmuls still retire work at half rate, so the warmup burst is
  sized only to bridge engine-boot (~6.85us) to x0/w1m0-ready (~9.5us);
  real waves start cold and warm up in place.
- exp tiles are two persistent buffers zeroed once during the fill (rows
  C..127 must be zero for the ones-matmul partition sum); v1 re-memset
  them every chunk.
"""

import math
from collections import deque

import ml_dtypes
import numpy as np

import concourse.bacc as bacc
import concourse.bass as bass
import concourse.mybir as mybir
import concourse.tile as tile
from concourse.bass import ds
from concourse.bass_utils import run_bass_kernel_spmd

N_CORES = 8
BF16 = ml_dtypes.bfloat16

K_SPLIT = 5  # x k-blocks 0:5 on sync (starts earlier), 5:8 on gpsimd (faster)
WU_BIG = 8  # 512-col warmup matmuls (cold, ~427ns each: covers the 3.4us HAM window)
WU_SMALL = 34  # 128-col warmup matmuls (post-warm ~60-80ns each, bridge to x0-ready ~13us)

_program_cache: dict[tuple, object] = {}


def _chunk_sizes(cap: int) -> list[int]:
    """[balanced <=512 chunks..., 256]: big chunks first (per-core DMA is
    only ~250-430GB/s aggregate, so the front must not need w1+x0+x1 all
    at once - a big chunk 0 buys the conveyor time), small chunk LAST so
    the post-stream tail (exp/norm/store) is short."""
    if cap <= 512:
        return [cap]
    cl = 256
    rest = cap - cl
    n = math.ceil(rest / 512)
    base, r = divmod(rest, n)
    return [base + (1 if i < r else 0) for i in range(n)] + [cl]


def _build_program(cap: int, F1: int, F2: int, C: int):
    key = (cap, F1, F2, C)
    if key in _program_cache:
        return _program_cache[key]

    assert F1 % 128 == 0 and F2 % 128 == 0
    K1 = F1 // 128
    M1 = F2 // 128
    K2 = F2 // 128
    assert C <= 128

    f32 = mybir.dt.float32
    bf16 = mybir.dt.bfloat16
    nc = bacc.Bacc(None, target_bir_lowering=False, debug=False)

    chunks = _chunk_sizes(cap)
    offs = [0]
    for cn in chunks:
        offs.append(offs[-1] + cn)
    n_chunks = len(chunks)

    x_d = [
        nc.dram_tensor(f"xt{ci}", [128, K1, cn], bf16, kind="ExternalInput")
        for ci, cn in enumerate(chunks)
    ]
    # w1: per m-block, K1 x 128 weight columns + 2 bf16 columns holding the
    # fp32 bias bit pattern (read via bitcast - exact).
    w1_d = nc.dram_tensor("w1", [128, M1, K1 * 128 + 2], bf16, kind="ExternalInput")
    # w2: K2 x C(=64) real weight columns + 2 bias columns. Layer 2 runs
    # as PAIRS of concurrent 64-wide matmuls in the two PE column-group
    # halves (tile_position (0,0) / (0,64)), so no 128-padding is needed;
    # a DVE add folds psum[0:64] + psum[64:128] into the logits.
    assert C == 64 and K2 % 2 == 0
    w2_d = nc.dram_tensor("w2", [128, K2 * C + 2], bf16, kind="ExternalInput")
    # Zero-padded 128-wide variant, used ONLY for the last chunk: its
    # whole L2+exp+store chain sits on the post-stream critical tail, and
    # the single-group form needs no PSUM-halves copy/add before EXP.
    w2p_d = nc.dram_tensor("w2p", [128, K2 * 128], bf16, kind="ExternalInput")
    # Unnormalized exp(logits); the softmax divide happens on the host
    # (exact fp32, replacing the device's approx-reciprocal) - this drops
    # the partition-sum matmul, reciprocal and multiply from the kernel.
    out_d = nc.dram_tensor("outT", [C, cap], bf16, kind="ExternalOutput")

    with tile.TileContext(nc) as tc:
        with (
            tc.tile_pool(name="const", bufs=1) as const_pool,
            tc.tile_pool(name="expp", bufs=2) as e_pool,
            tc.tile_pool(name="xin", bufs=2) as x_pool,
            tc.tile_pool(name="h", bufs=2 * M1 + 2) as h_pool,
            tc.tile_pool(name="lg", bufs=2) as l_pool,
            tc.tile_pool(name="ph", bufs=5, space="PSUM") as ph_pool,
            tc.tile_pool(name="pl", bufs=2, space="PSUM") as pl_pool,
        ):
            # Scalar ring (HWDGE): w1 m0 first - nothing descriptor-dense
            # ahead of it - then the x0 k5:8 tail. w1 m1..m3 and w2 are
            # emitted between chunk-0 waves below.
            w1_sb = const_pool.tile([128, M1, K1 * 128 + 2], bf16)
            nc.scalar.dma_start(w1_sb[:, 0, :], w1_d[:, 0, :])
            w2_sb = const_pool.tile([128, K2 * C + 2], bf16)
            w2p_sb = const_pool.tile([128, K2 * 128], bf16)

            # Warmup operand memset FIRST on the gpsimd queue (before its
            # DMA half) so the warmup starts as soon as the engine boots.
            wu_x = const_pool.tile([128, 512], bf16)
            nc.gpsimd.memset(wu_x[:], 0.0)

            # x0 halves on the sync + gpsimd rings (v1-proven pacing: the
            # per-core DMA aggregate is only ~210GB/s, so the scalar ring
            # keeps the whole w1/w2 stream to itself while x rides the
            # other two rings, one chunk in flight at a time).
            xt = []
            t = x_pool.tile([128, K1, chunks[0]], bf16, tag="xt", name="xt0")
            nc.sync.dma_start(t[:, :K_SPLIT, :], x_d[0][:, :K_SPLIT, :])
            nc.gpsimd.dma_start(t[:, K_SPLIT:, :], x_d[0][:, K_SPLIT:, :])
            xt.append(t)



            # Warmup: bridge engine-boot (~6.85us) to first-data (~9.5us).
            # Cold matmuls retire real work at half rate, so undershoot
            # beats overshoot; HAM goes warm at first-busy+3.4us no matter
            # what we do here.
            for i in range(WU_BIG):
                wu_ps = ph_pool.tile([128, 512], f32, tag="ph", name=f"wu{i}")
                nc.tensor.matmul(
                    wu_ps[:], wu_x[:, :128], wu_x[:], start=True, stop=True
                )
            for i in range(WU_SMALL):
                wu_ps = ph_pool.tile([128, 128], f32, tag="ph", name=f"wv{i}")
                nc.tensor.matmul(
                    wu_ps[:], wu_x[:, :128], wu_x[:, :128], start=True, stop=True
                )

            b1_ap = [
                w1_sb[:, m, K1 * 128 : K1 * 128 + 2].bitcast(f32)
                for m in range(M1)
            ]
            b2_ap = w2_sb[0:C, K2 * C : K2 * C + 2].bitcast(f32)

            stages: deque = deque()

            def stage_l2(ci: int, cn: int, ht: list):
                pl = pl_pool.tile([128, cn], f32, tag="pl")
                if ci == n_chunks - 1:
                    # Post-stream critical tail: single-group L2 on the
                    # zero-padded w2 so EXP can read PSUM directly - no
                    # PSUM-halves copy/add in the chain.
                    for k in range(K2):
                        nc.tensor.matmul(
                            pl[:],
                            w2p_sb[:, k * 128 : (k + 1) * 128],
                            ht[k][:],
                            start=(k == 0),
                            stop=(k == K2 - 1),
                        )
                    exp_src = pl[0:C, :]
                else:
                    # Mid-stream: the two PE column-group halves run each
                    # k2-pair CONCURRENTLY (~2x), and the fold-up runs on
                    # the idle DVE a full wave ahead of the store.
                    for kp in range(K2 // 2):
                        ka, kb = 2 * kp, 2 * kp + 1
                        nc.tensor.matmul(
                            pl[0:C, :],
                            w2_sb[:, ka * C : (ka + 1) * C],
                            ht[ka][:],
                            start=(kp == 0),
                            stop=(kp == K2 // 2 - 1),
                            tile_position=(0, 0),
                        )
                        nc.tensor.matmul(
                            pl[C : 2 * C, :],
                            w2_sb[:, kb * C : (kb + 1) * C],
                            ht[kb][:],
                            start=(kp == 0),
                            stop=(kp == K2 // 2 - 1),
                            tile_position=(0, C),
                        )
                    # DVE may read only one PSUM operand per instruction:
                    # copy one half out, then add in place.
                    lg = l_pool.tile([C, cn], f32, tag="lg")
                    nc.vector.tensor_copy(lg[:], pl[C : 2 * C, :])
                    nc.vector.tensor_add(lg[:], lg[:], pl[0:C, :])
                    exp_src = lg[:]
                expt = e_pool.tile([C, cn], bf16, tag="expt")
                nc.scalar.activation(
                    expt[:],
                    exp_src,
                    mybir.ActivationFunctionType.Exp,
                    bias=b2_ap,
                )
                # Sync HWDGE, not gpsimd SWDGE: SWDGE descriptor
                # generation costs ~1us and the final store sits on the
                # critical tail.
                nc.sync.dma_start(out_d[:, ds(offs[ci], cn)], expt[:])

            for ci, cn in enumerate(chunks):
                ht = []
                for m in range(M1):
                    ph = ph_pool.tile([128, cn], f32, tag="ph")
                    for k in range(K1):
                        nc.tensor.matmul(
                            ph[:],
                            w1_sb[:, m, k * 128 : (k + 1) * 128],
                            xt[ci][:, k, :],
                            start=(k == 0),
                            stop=(k == K1 - 1),
                        )
                    hm = h_pool.tile([128, cn], bf16, tag="ht")
                    nc.scalar.activation(
                        hm[:],
                        ph[:],
                        mybir.ActivationFunctionType.Relu,
                        bias=b1_ap[m],
                    )
                    ht.append(hm)
                    if ci == 0 and m + 1 < M1:
                        # Emitted AFTER wave m so wave m+1 (not wave m)
                        # carries the wait for this transfer.
                        nc.scalar.dma_start(
                            w1_sb[:, m + 1, :], w1_d[:, m + 1, :]
                        )
                    if ci == 0 and m == 2:
                        nc.scalar.dma_start(w2_sb[:], w2_d[:])
                    if ci == 0 and m == 3:
                        nc.scalar.dma_start(w2p_sb[:], w2p_d[:])
                    if stages:
                        stages.popleft()()
                # Next chunk's x, emitted AFTER this chunk's waves so this
                # chunk's matmuls don't wait on it (a consumer waits on
                # every DMA emitted earlier on the same ring). x1/x2 ride
                # the two HWDGE conveyors right behind the fill; x3/x4
                # move their k0:5 bulk to gpsimd (by then its SWDGE lag is
                # harmless and the HWDGE rings carry stores + k5:8).
                # Prefetch next chunk's x AFTER this chunk's waves: a
                # consumer waits on every DMA emitted earlier on the same
                # queue, so emitting the prefetch first would make THIS
                # chunk's matmuls wait for the NEXT chunk's transfer.
                nxt = ci + 1
                if nxt < n_chunks:
                    cnn = chunks[nxt]
                    t = x_pool.tile(
                        [128, K1, cnn], bf16, tag="xt", name=f"xt{nxt}"
                    )
                    # Gate the prefetch's ISSUE (not just its consumers)
                    # behind the current chunk's data: this corner copy
                    # reads chunk ci (both DMA halves) and writes a corner
                    # of chunk ci+1's tile, so the WAW dependency stops the
                    # next D2D from interleaving its descriptors with the
                    # in-flight transfer and halving its bandwidth. The DMA
                    # overwrites the corner immediately.
                    nc.vector.tensor_copy(
                        t[0:1, :, 0:1], xt[ci][0:1, :, 0:1]
                    )
                    nc.sync.dma_start(
                        t[:, :K_SPLIT, :], x_d[nxt][:, :K_SPLIT, :]
                    )
                    nc.gpsimd.dma_start(
                        t[:, K_SPLIT:, :], x_d[nxt][:, K_SPLIT:, :]
                    )
                    xt.append(t)
                stages.append(lambda ci=ci, cn=cn, ht=ht: stage_l2(ci, cn, ht))
            while stages:
                stages.popleft()()

    nc.compile()
    _program_cache[key] = nc
    return nc


def _pack_bias_cols(b: np.ndarray) -> np.ndarray:
    """fp32 [..., n] -> bf16 [..., n, 2] bit-pattern split."""
    a = np.ascontiguousarray(b, dtype="<f4")
    return a.view(np.uint16).view(BF16).reshape(*a.shape, 2)


def kernel(domain, x, W1, b1, W2, b2):
    domain = np.asarray(domain)
    x = np.ascontiguousarray(np.asarray(x, dtype=np.float32))
    W1 = np.asarray(W1, dtype=np.float32)
    b1 = np.asarray(b1, dtype=np.float32)
    W2 = np.asarray(W2, dtype=np.float32)
    b2 = np.asarray(b2, dtype=np.float32)

    B, F1 = x.shape
    E, _, F2 = W1.shape
    C = W2.shape[2]
    K1 = F1 // 128
    K2 = F2 // 128
    M1 = F2 // 128
    assert E == N_CORES

    xb = x.astype(BF16)
    W1b = W1.astype(BF16)
    W2b = W2.astype(BF16)

    idx = [np.nonzero(domain == e)[0] for e in range(E)]
    counts = [len(i) for i in idx]
    cap = max(512, max(counts))
    chunks = _chunk_sizes(cap)

    nc = _build_program(cap, F1, F2, C)

    in_maps = []
    for e in range(E):
        xT = np.zeros((F1, cap), BF16)
        xT[:, : counts[e]] = xb[idx[e]].T
        # [F1, cap] -> [128, K1, cap] SBUF tile layout.
        xT4 = xT.reshape(K1, 128, cap).transpose(1, 0, 2)

        w1p = np.zeros((128, M1, K1 * 128 + 2), BF16)
        w1p[:, :, : K1 * 128] = (
            W1b[e].reshape(K1, 128, M1, 128).transpose(1, 2, 0, 3)
            .reshape(128, M1, K1 * 128)
        )
        # b1 [F2] -> [128 partitions (f2-within), M1] fp32 bits.
        w1p[:, :, K1 * 128 :] = _pack_bias_cols(b1[e].reshape(M1, 128).T)

        w2blk = W2b[e].reshape(K2, 128, C).transpose(1, 0, 2)
        w2p = np.zeros((128, K2 * C + 2), BF16)
        w2p[:, : K2 * C] = w2blk.reshape(128, K2 * C)
        w2p[:C, K2 * C :] = _pack_bias_cols(b2[e])
        w2pp = np.zeros((128, K2, 128), BF16)
        w2pp[:, :, :C] = w2blk

        m = {
            "w1": np.ascontiguousarray(w1p),
            "w2": np.ascontiguousarray(w2p),
            "w2p": np.ascontiguousarray(w2pp.reshape(128, K2 * 128)),
        }
        n0 = 0
        for ci, cn in enumerate(chunks):
            m[f"xt{ci}"] = np.ascontiguousarray(xT4[:, :, n0 : n0 + cn])
            n0 += cn
        in_maps.append(m)

    res = run_bass_kernel_spmd(nc, in_maps, core_ids=list(range(N_CORES)))

    out = np.empty((B, C), np.float32)
    for e in range(E):
        # Device returns unnormalized exp(logits); exact softmax divide
        # here (replaces the device-side approx-reciprocal).
        et = res.results[e]["outT"][:, : counts[e]].T.astype(np.float32)
        out[idx[e]] = et / et.sum(axis=1, keepdims=True)
    return out
